# revision 43
# baseline (speedup 1.0000x reference)
"""Trainium2 Bass kernel for nn_NoFoDifformer_FourierKAN (8-core SPMD), v4.

Sharding: u and nodes row-wise across 8 cores (1250 rows each). The z = u^T h
partial sums are all-reduced per column-chunk (bf16); the [d,d] Gram matrix of
the normalized activations rides inside the first chunk's all-reduce. Small
weights are pre-folded and packed on the HOST (LayerNorm affines folded into
projections, lambda and the FourierKAN filter evaluated in numpy) so the
device preamble is a handful of panel DMAs. Per-core outputs are [d, n_loc]
(transposed) and transposed+concatenated on the host.

Scheduling notes (engine FIFOs are in-order; a blocked DMA trigger blocks
everything behind it on that queue, so queue assignment is load-bearing):
- sync queue: xT, all u quarter-tiles, then all uT tiles, zc readbacks, out
  blocks. uT triggers drain only after the u triggers, so uT prefetch can
  never race the pass1-feeding u stream.
- scalar queue: activations + pass1 PSUM->staging casts (no uT triggers ->
  the LN Sqrt chain can never deadlock against uT pool recycling).
- gpsimd queue: weight panels, p1 staging writes, AR triggers. AR triggers
  serialize on the previous AR's completion, keeping the chain dense.
- ASCENDING chunk plan [2048, 4096, 3856]: the small chunk 0 puts the first
  AllReduce right at the end of the startup barrier (the AR chain is the
  critical path: ~20 GB/s alg ring bandwidth); the big middle ARs hide
  pass2/uT streaming; lnT+gram are emitted inside pass1(0) so the Gram
  rides chunk 0's AR with minimal doorbell delay.
- attention epilogue (watt/sT/haT) is emitted after the pass1 stages and
  accumulates commutatively into haT (initialized as a copy of hT).
"""

import numpy as np

N_FULL = 10000
NF_FULL = 512
D = 128
CORES_FULL = 8
LAMBDA_INIT = 0.2
GEXT = 256  # extra AR columns on chunk 0 for (G | s)
CH_PLAN = [2048, 4096, 3856]


def _splits(total, step):
    return [(o, min(step, total - o)) for o in range(0, total, step)]


def build_kernel(N=N_FULL, NF=NF_FULL, CORES=CORES_FULL):
    import concourse.bacc as bacc
    import concourse.tile as tile
    from concourse import mybir
    from concourse.masks import make_identity
    from contextlib import ExitStack

    dt = mybir.dt
    f32 = dt.float32
    bf16 = dt.bfloat16
    AF = mybir.ActivationFunctionType
    ALU = mybir.AluOpType

    NLOC = N // CORES                   # 1250
    ROWS = _splits(NLOC, 128)           # 9x128 + 98
    NT = len(ROWS)
    KX = NF // 128
    assert sum(CH_PLAN) == N
    assert all(w % 128 == 0 for w in CH_PLAN[:-1])
    CHUNKS = []
    off = 0
    for w in CH_PLAN:
        CHUNKS.append((off, w))
        off += w
    NCH = len(CHUNKS)
    CHMAX = max(CH_PLAN)
    NSUB = (N + 127) // 128             # 79
    BLK = _splits(NLOC, 512)            # [d, NLOC] op blocks
    rg = [list(range(CORES))]
    shared_space = "Shared" if CORES > 4 else "Local"

    nc = bacc.Bacc("TRN2", target_bir_lowering=False, debug=False,
                   num_devices=CORES)

    # ---------------- DRAM I/O ----------------
    t_u = nc.dram_tensor("u", [NLOC, N], bf16, kind="ExternalInput")
    t_uT = nc.dram_tensor("uT", [N, NLOC], bf16, kind="ExternalInput")
    t_xT = nc.dram_tensor("xT", [NF, NLOC], bf16, kind="ExternalInput")
    t_wb = nc.dram_tensor("wb", [128, 13 * 128], bf16, kind="ExternalInput")
    t_colw = nc.dram_tensor("colw", [128, 8], f32, kind="ExternalInput")
    t_colb = nc.dram_tensor("colb", [128, 2], bf16, kind="ExternalInput")
    t_rowb = nc.dram_tensor("rowb", [1, 4 * 128], bf16, kind="ExternalInput")
    t_roww = nc.dram_tensor("roww", [1, 128], f32, kind="ExternalInput")
    t_ne = nc.dram_tensor("ne", [128, NSUB], f32, kind="ExternalInput")
    t_out = nc.dram_tensor("out", [D, NLOC], f32, kind="ExternalOutput")

    with tile.TileContext(nc) as tc, ExitStack() as ctx:
        wpool = ctx.enter_context(tc.tile_pool(name="wpool", bufs=1))
        rowtmp = ctx.enter_context(tc.tile_pool(name="rowtmp", bufs=3))
        ustream = ctx.enter_context(tc.tile_pool(name="ustream", bufs=16))
        uTp = ctx.enter_context(tc.tile_pool(name="uTp", bufs=19))
        zcp = ctx.enter_context(tc.tile_pool(name="zcp", bufs=1))
        z16p = ctx.enter_context(tc.tile_pool(name="z16p", bufs=1))
        p1sbp = ctx.enter_context(tc.tile_pool(name="p1sbp", bufs=1))
        dram = ctx.enter_context(tc.tile_pool(name="dram", bufs=1, space="DRAM"))
        ps_p1 = ctx.enter_context(tc.tile_pool(name="ps_p1", bufs=2, space="PSUM"))
        ps_p2 = ctx.enter_context(tc.tile_pool(name="ps_p2", bufs=3, space="PSUM"))
        ps_mm = ctx.enter_context(tc.tile_pool(name="ps_mm", bufs=1, space="PSUM"))
        ps_t = ctx.enter_context(tc.tile_pool(name="ps_t", bufs=2, space="PSUM"))

        def p1_tile(w):
            return ps_p1.tile([128, 512], f32, tag="p1",
                              name=f"p1_{nc.next_id()}")[:, :w]

        def p2_tile(w):
            return ps_p2.tile([128, 512], f32, tag="p2",
                              name=f"p2_{nc.next_id()}")[:, :w]

        def mm_tile(p, w):
            return ps_mm.tile([128, 512], f32, tag="mmp",
                              name=f"mm_{nc.next_id()}")[:p, :w]

        def tb_tile(p, w):
            return ps_t.tile([128, 128], bf16, tag="pstb",
                             name=f"pstb_{nc.next_id()}")[:p, :w]

        def wtile(shape, dtype, name):
            return wpool.tile(shape, dtype, tag=name, name=name)

        def rtile(shape, dtype, tag):
            return rowtmp.tile(shape, dtype, tag=tag,
                               name=f"{tag}_{nc.next_id()}")

        def T(out_psum, in_sbuf, identity):
            nc.tensor.matmul(out_psum, in_sbuf, identity, is_transpose=True)

        # ================= constants & weights =================
        identb = wtile([128, 128], bf16, "identb")
        make_identity(nc, identb[:])
        ones_row_b = wtile([1, 128], bf16, "ones_row_b")
        nc.vector.memset(ones_row_b[:], 1.0)
        ones_col_b = wtile([128, 1], bf16, "ones_col_b")
        nc.vector.memset(ones_col_b[:], 1.0)
        oinv_col_b = wtile([128, 1], bf16, "oinv_col_b")
        nc.vector.memset(oinv_col_b[:], 1.0 / 128.0)
        eps_col = wtile([128, 1], f32, "eps_col")
        nc.vector.memset(eps_col[:], 1e-5)

        wb = wtile([128, 13 * 128], bf16, "wb")
        nc.gpsimd.dma_start(out=wb[:], in_=t_wb[:])
        colw = wtile([128, 8], f32, "colw")
        nc.gpsimd.dma_start(out=colw[:], in_=t_colw[:])
        colb = wtile([128, 2], bf16, "colb")
        nc.gpsimd.dma_start(out=colb[:], in_=t_colb[:])
        rowb = wtile([1, 4 * 128], bf16, "rowb")
        nc.gpsimd.dma_start(out=rowb[:], in_=t_rowb[:])
        roww = wtile([1, 128], f32, "roww")
        nc.gpsimd.dma_start(out=roww[:], in_=t_roww[:])
        ne = wtile([128, NSUB], f32, "ne")
        nc.gpsimd.dma_start(out=ne[:], in_=t_ne[:])

        def P(i):  # weight panel i of wb
            return wb[:, i * 128:(i + 1) * 128]
        few2b = P(4)
        Wk1b, Wk2b, Wvb = P(5), P(6), P(7)
        Wq1Tb, Wq2Tsb = P(8), P(9)
        Wob, W1pb, f2wb = P(10), P(11), P(12)
        feb1_c = colw[:, 0:1]
        feb2_c = colw[:, 1:2]
        bo_c = colw[:, 2:3]
        b1p_c = colw[:, 3:4]
        f2b_c = colw[:, 4:5]

        # xT into SBUF (4 partition k-tiles)
        xT4 = wtile([128, KX, NLOC], bf16, "xT4")
        for kt in range(KX):
            nc.sync.dma_start(out=xT4[:, kt, :],
                              in_=t_xT[kt * 128:(kt + 1) * 128, :])

        # ---------- u streaming loads (quarter tiles, sync queue) ----------
        u_tiles = {}

        def emit_u_loads(c):
            co, cw = CHUNKS[c]
            tiles = {}
            for q, (qo, qw) in enumerate(_splits(cw, 1024)):
                for r, (ro, rw) in enumerate(ROWS):
                    ut = ustream.tile([128, 1024], bf16, tag="u",
                                      name=f"u{c}_{q}_{r}")[:rw, :qw]
                    nc.sync.dma_start(
                        out=ut, in_=t_u[ro:ro + rw, co + qo:co + qo + qw])
                    tiles[(q, r)] = ut
            u_tiles[c] = tiles

        uT_tiles = {}

        def emit_uT_loads(c, eng=None):
            # alternate the two HWDGE queues so uT transfers stream in
            # parallel instead of serializing on one hardware queue
            co, cw = CHUNKS[c]
            tl = []
            for t, (so, sw) in enumerate(_splits(cw, 128)):
                uTt = uTp.tile([128, NLOC], bf16, tag="uT",
                               name=f"uTl{c}_{t}")[:sw]
                e = eng or (nc.sync if t % 2 == 0 else nc.scalar)
                e.dma_start(out=uTt, in_=t_uT[co + so:co + so + sw, :])
                tl.append(uTt)
            uT_tiles[c] = tl

        for _c in range(NCH):
            emit_u_loads(_c)

        # ================= phase A: feature encoder (transposed) ==========
        hT = wtile([128, NLOC], f32, "hT")
        hTb = wtile([128, NLOC], bf16, "hTb")
        h16 = wtile([128, NT, D], bf16, "h16")
        for go, gw in BLK:
            psh1 = p2_tile(gw)
            for kt in range(KX):
                nc.tensor.matmul(psh1, P(kt), xT4[:, kt, go:go + gw],
                                 start=(kt == 0), stop=(kt == KX - 1))
            h1t = rtile([128, 512], bf16, "h1t")[:, :gw]
            nc.scalar.activation(h1t, psh1, AF.Relu, bias=feb1_c[:])
            pshT = p2_tile(gw)
            nc.tensor.matmul(pshT, few2b, h1t)
            nc.vector.tensor_scalar(hT[:, go:go + gw], pshT,
                                    scalar1=feb2_c, scalar2=None, op0=ALU.add)
            nc.scalar.activation(hTb[:, go:go + gw], hT[:, go:go + gw],
                                 AF.Copy)
        for r, (ro, rw) in enumerate(ROWS):
            pst = tb_tile(rw, 128)
            T(pst, hTb[:, ro:ro + rw], identb[:])
            nc.vector.tensor_copy(h16[:rw, r, :], pst)

        # ---------- transposed-layout LayerNorm helper ----------
        def lnT(x_sb, out_bf, pfx, xb=None):
            if xb is None:
                xb = wpool.tile([128, NLOC], bf16, tag="ln_xb",
                                name=f"{pfx}_xb")
                nc.scalar.activation(xb[:], x_sb[:], AF.Copy)
            x2b = wpool.tile([128, NLOC], bf16, tag="ln_x2b",
                             name=f"{pfx}_x2b")
            nc.vector.tensor_mul(x2b[:], x_sb[:], x_sb[:])

            def frow(tag, dt_):
                return rowtmp.tile([1, NLOC], dt_, tag=tag, bufs=2,
                                   name=f"{tag}_{nc.next_id()}")
            mrow = frow("ln_m", f32)
            rsrow = frow("ln_r", f32)
            for bo, bw in BLK:
                psm = mm_tile(1, bw)
                nc.tensor.matmul(psm, oinv_col_b[:], xb[:, bo:bo + bw])
                nc.vector.tensor_copy(mrow[:, bo:bo + bw], psm)
                psq_ = mm_tile(1, bw)
                nc.tensor.matmul(psq_, oinv_col_b[:], x2b[:, bo:bo + bw])
                nc.vector.tensor_mul(rsrow[:, bo:bo + bw],
                                     mrow[:, bo:bo + bw], mrow[:, bo:bo + bw])
                nc.vector.tensor_sub(rsrow[:, bo:bo + bw], psq_,
                                     rsrow[:, bo:bo + bw])         # var
            nc.scalar.activation(rsrow[:], rsrow[:], AF.Sqrt,
                                 bias=eps_col[:1])
            nc.vector.reciprocal(rsrow[:], rsrow[:])               # 1/sqrt
            m_b = frow("ln_mb", bf16)
            nc.vector.tensor_copy(m_b[:], mrow[:])
            rs_b = frow("ln_rb", bf16)
            nc.vector.tensor_copy(rs_b[:], rsrow[:])
            for bo, bw in BLK:
                psM = p2_tile(bw)
                nc.tensor.matmul(psM, ones_row_b[:], m_b[:, bo:bo + bw])
                psR = p2_tile(bw)
                nc.tensor.matmul(psR, ones_row_b[:], rs_b[:, bo:bo + bw])
                dtmp = rowtmp.tile([128, 512], f32, tag="btmp", bufs=2,
                                   name=f"lnd_{nc.next_id()}")[:, :bw]
                nc.vector.tensor_sub(dtmp, x_sb[:, bo:bo + bw], psM)
                nc.vector.tensor_mul(out_bf[:, bo:bo + bw], dtmp, psR)

        hnTb = wtile([128, NLOC], bf16, "hnTb")
        gram_sb = wtile([128, GEXT], bf16, "gram_sb")

        def emit_lngram():
            # LN(h) + Gram: G = hn^T hn, s = hn^T 1 (into gram_sb bf16)
            lnT(hT, hnTb, "hn", xb=hTb)
            psGS = ps_mm.tile([128, 512], f32, tag="mmp", name="psGS")
            for r, (ro, rw) in enumerate(ROWS):
                pst = tb_tile(rw, 128)
                T(pst, hnTb[:, ro:ro + rw], identb[:])
                hn_r = rtile([128, 128], bf16, "hn_r")[:rw]
                nc.vector.tensor_copy(hn_r, pst)
                nc.tensor.matmul(psGS[:, 0:128], hn_r, hn_r,
                                 start=(r == 0), stop=(r == NT - 1))
                nc.tensor.matmul(psGS[:1, 128:256], ones_col_b[:rw], hn_r,
                                 start=(r == 0), stop=(r == NT - 1))
            nc.vector.tensor_copy(gram_sb[:, 0:128], psGS[:, 0:128])
            nc.vector.tensor_copy(gram_sb[:1, 128:256], psGS[:1, 128:256])

        # ---------- DRAM staging ----------
        p1_in, p1_out = [], []
        for c, (co, cw) in enumerate(CHUNKS):
            w = cw + (GEXT if c == 0 else 0)
            p1_in.append(dram.tile([128, w], bf16, tag=f"p1in{c}",
                                   name=f"p1in{c}"))
            p1_out.append(dram.tile([128, w], bf16, tag=f"p1out{c}",
                                    name=f"p1out{c}", addr_space=shared_space))

        haT = wtile([128, NLOC], f32, "haT")
        sT = wtile([128, NLOC], f32, "sT")
        sTb = wpool.tile([128, NLOC], bf16, tag="hTb", name="sTb")
        aTb = wpool.tile([128, NLOC], bf16, tag="hnTb", name="aTb")

        def emit_pass1(c, mid=None):
            co, cw = CHUNKS[c]
            ut = u_tiles.pop(c)
            w = cw + (GEXT if c == 0 else 0)
            p1sb = p1sbp.tile([128, CHMAX + GEXT], bf16, tag="p1sb",
                              name=f"p1sb{c}")[:, :w]
            for q, (qo, qw) in enumerate(_splits(cw, 1024)):
                blocks = _splits(qw, 512)
                ps1 = [p1_tile(bw) for _, bw in blocks]
                for r, (ro, rw) in enumerate(ROWS):
                    for bi, (bo, bw) in enumerate(blocks):
                        nc.tensor.matmul(ps1[bi], h16[:rw, r, :],
                                         ut[(q, r)][:rw, bo:bo + bw],
                                         start=(r == 0), stop=(r == NT - 1))
                if q == 0 and mid is not None:
                    mid()
                for bi, (bo, bw) in enumerate(blocks):
                    nc.scalar.activation(p1sb[:, qo + bo:qo + bo + bw],
                                         ps1[bi], AF.Copy)
            if c == 0:
                nc.scalar.activation(p1sb[:, cw:cw + GEXT], gram_sb[:],
                                     AF.Copy)
            nc.gpsimd.dma_start(out=p1_in[c][:], in_=p1sb)
            nc.gpsimd.collective_compute(
                "AllReduce", ALU.add, replica_groups=rg,
                ins=[p1_in[c].opt()], outs=[p1_out[c].opt()])

        def emit_pass2(c):
            co, cw = CHUNKS[c]
            subs = _splits(cw, 128)
            zc = zcp.tile([128, CHMAX], bf16, tag="zc",
                          name=f"zc_{c}")[:, :cw]
            nc.scalar.dma_start(out=zc, in_=p1_out[c][:, :cw])
            z16 = z16p.tile([128, (CHMAX + 127) // 128, D], bf16, tag="z16",
                            name=f"z16_{c}")
            for t, (so, sw) in enumerate(subs):
                psz = tb_tile(sw, 128)
                T(psz, zc[:, so:so + sw], identb[:])
                gidx = (co + so) // 128
                nc.vector.tensor_scalar(z16[:sw, t, :], psz,
                                        scalar1=ne[:sw, gidx:gidx + 1],
                                        scalar2=None, op0=ALU.mult)
            uTc = uT_tiles.pop(c)
            ps2 = [p2_tile(iw) for _, iw in BLK]
            for t, (so, sw) in enumerate(subs):
                for ib, (io, iw) in enumerate(BLK):
                    nc.tensor.matmul(ps2[ib], z16[:sw, t, :],
                                     uTc[t][:sw, io:io + iw],
                                     start=(t == 0), stop=(t == len(subs) - 1))
            for ib, (io, iw) in enumerate(BLK):
                nc.vector.tensor_add(haT[:, io:io + iw],
                                     haT[:, io:io + iw], ps2[ib])

        def emit_att():
            # post-AR0 attention path: gram -> k1v/k2v -> Watt -> sT -> aT -> haT
            co0, cw0 = CHUNKS[0]
            gkv = wtile([128, GEXT], bf16, "gkv")
            nc.scalar.dma_start(out=gkv[:], in_=p1_out[0][:, cw0:cw0 + GEXT])
            G_b = gkv[:, 0:128]
            s_row = gkv[:1, 128:256]
            psc = tb_tile(128, 1)
            T(psc, s_row, identb[:1, :1])
            s_col = rtile([128, 1], bf16, "s_col")
            nc.vector.tensor_copy(s_col[:], psc)
            # X1 = G Wv + s (.) bv   (shared by k1v and k2v)
            psX = mm_tile(128, 128)
            nc.tensor.matmul(psX, G_b, Wvb, start=True, stop=False)
            nc.tensor.matmul(psX, s_row, rowb[:1, 256:384], start=False,
                             stop=True)
            X1b = wtile([128, 128], bf16, "X1b")
            nc.vector.tensor_copy(X1b[:], psX)
            # rrow = s^T Wv + N bv
            psr = mm_tile(1, 128)
            nc.tensor.matmul(psr, s_col[:], Wvb)
            rrow = rtile([1, 128], f32, "rrow")
            nc.vector.tensor_add(rrow[:], psr, roww[:1])
            rrow_b = rtile([1, 128], bf16, "rrow_b")
            nc.vector.tensor_copy(rrow_b[:], rrow[:])
            kvs = []
            for i, Wk in ((0, Wk1b), (1, Wk2b)):
                psK = mm_tile(128, 128)
                nc.tensor.matmul(psK, Wk, X1b[:], start=True, stop=False)
                nc.tensor.matmul(psK, rowb[:1, i * 128:(i + 1) * 128],
                                 rrow_b[:], start=False, stop=True)
                kv = wtile([128, 128], bf16, f"k{i+1}v_b")
                nc.vector.tensor_copy(kv[:], psK)
                kvs.append(kv)
            psW = mm_tile(128, 128)
            nc.tensor.matmul(psW, Wq1Tb, kvs[0][:], start=True, stop=False)
            nc.tensor.matmul(psW, Wq2Tsb, kvs[1][:], start=False, stop=True)
            Wattb = wtile([128, D], bf16, "Wattb")
            nc.vector.tensor_copy(Wattb[:], psW)
            psB = mm_tile(128, 1)
            nc.tensor.matmul(psB, kvs[0][:], colb[:, 0:1], start=True,
                             stop=False)
            nc.tensor.matmul(psB, kvs[1][:], colb[:, 1:2], start=False,
                             stop=True)
            batt_c = wtile([128, 1], f32, "batt_c")
            nc.vector.tensor_copy(batt_c[:], psB)
            # sT = Watt^T @ hnT + batt  (transposed layout)
            for bo, bw in BLK:
                pss = p2_tile(bw)
                nc.tensor.matmul(pss, Wattb[:], hnTb[:, bo:bo + bw])
                nc.vector.tensor_scalar(sT[:, bo:bo + bw], pss,
                                        scalar1=batt_c[:], scalar2=None,
                                        op0=ALU.add)
                nc.vector.tensor_copy(sTb[:, bo:bo + bw], sT[:, bo:bo + bw])
            lnT(sT, aTb, "s", xb=sTb)
            # haT += Wo'^T @ aT + bo   (commutative accumulation)
            for bo, bw in BLK:
                psa = p2_tile(bw)
                nc.tensor.matmul(psa, Wob, aTb[:, bo:bo + bw])
                atmp = rowtmp.tile([128, 512], f32, tag="btmp", bufs=2,
                                   name=f"atmp_{nc.next_id()}")[:, :bw]
                nc.vector.tensor_scalar(atmp, psa, scalar1=bo_c,
                                        scalar2=None, op0=ALU.add)
                nc.vector.tensor_add(haT[:, bo:bo + bw],
                                     haT[:, bo:bo + bw], atmp)

        # ---- pipeline ----
        emit_pass1(0, mid=emit_lngram)
        for _c in range(1, NCH):
            emit_pass1(_c)
        for _c in range(NCH):
            emit_uT_loads(_c)
        # haT starts as a copy of hT; att and pass2 then += into it
        nc.vector.tensor_copy(haT[:], hT[:])
        emit_att()           # runs in the gram-AR -> AR0 gap (Tensor idle)
        for _c in range(NCH):
            emit_pass2(_c)

        # ======= final epilogue: fused per-block LN+FFN, early out DMA =====
        outT = sT  # sT is dead after lnT(sT); reuse its buffer
        fxb = wpool.tile([128, NLOC], bf16, tag="hTb", name="fxb")
        fx2 = wpool.tile([128, NLOC], bf16, tag="ln_x2b", name="fx2")
        mrow = rowtmp.tile([1, NLOC], f32, tag="ep_m", bufs=1, name="ep_m")
        vrow = rowtmp.tile([1, NLOC], f32, tag="ep_v", bufs=1, name="ep_v")
        for bo, bw in BLK:
            nc.scalar.activation(fxb[:, bo:bo + bw], haT[:, bo:bo + bw],
                                 AF.Copy)
            nc.vector.tensor_mul(fx2[:, bo:bo + bw], haT[:, bo:bo + bw],
                                 haT[:, bo:bo + bw])
            psm = mm_tile(1, bw)
            nc.tensor.matmul(psm, oinv_col_b[:], fxb[:, bo:bo + bw])
            nc.vector.tensor_copy(mrow[:, bo:bo + bw], psm)
            psq_ = mm_tile(1, bw)
            nc.tensor.matmul(psq_, oinv_col_b[:], fx2[:, bo:bo + bw])
            nc.vector.tensor_mul(vrow[:, bo:bo + bw], mrow[:, bo:bo + bw],
                                 mrow[:, bo:bo + bw])
            nc.vector.tensor_sub(vrow[:, bo:bo + bw], psq_,
                                 vrow[:, bo:bo + bw])
        nc.scalar.activation(vrow[:], vrow[:], AF.Sqrt, bias=eps_col[:1])
        nc.vector.reciprocal(vrow[:], vrow[:])
        m_bf = rowtmp.tile([1, NLOC], bf16, tag="ep_mb", bufs=1, name="ep_mb")
        nc.vector.tensor_copy(m_bf[:], mrow[:])
        rs_bf = rowtmp.tile([1, NLOC], bf16, tag="ep_rb", bufs=1,
                            name="ep_rb")
        nc.vector.tensor_copy(rs_bf[:], vrow[:])
        stats = [(m_bf[:, bo:bo + bw], rs_bf[:, bo:bo + bw])
                 for bo, bw in BLK]
        for (bo, bw), (m_b, rs_b) in zip(BLK, stats):
            psM = p2_tile(bw)
            nc.tensor.matmul(psM, ones_row_b[:], m_b)
            psR = p2_tile(bw)
            nc.tensor.matmul(psR, ones_row_b[:], rs_b)
            dtmp = rowtmp.tile([128, 512], f32, tag="btmp", bufs=2,
                               name=f"lnd_{nc.next_id()}")[:, :bw]
            nc.vector.tensor_sub(dtmp, haT[:, bo:bo + bw], psM)
            fb_ = rtile([128, 512], bf16, "fb")[:, :bw]
            nc.vector.tensor_mul(fb_, dtmp, psR)
            psg_ = p2_tile(bw)
            nc.tensor.matmul(psg_, W1pb, fb_)
            gb_ = rtile([128, 512], bf16, "gb")[:, :bw]
            nc.scalar.activation(gb_, psg_, AF.Gelu, bias=b1p_c[:])
            pso_ = mm_tile(128, bw)
            nc.tensor.matmul(pso_, f2wb, gb_)
            otmp = rowtmp.tile([128, 512], f32, tag="btmp", bufs=2,
                               name=f"otmp_{nc.next_id()}")[:, :bw]
            nc.vector.tensor_scalar(otmp, pso_, scalar1=f2b_c,
                                    scalar2=None, op0=ALU.add)
            nc.vector.tensor_add(outT[:, bo:bo + bw],
                                 haT[:, bo:bo + bw], otmp)
            nc.sync.dma_start(out=t_out[:, bo:bo + bw],
                              in_=outT[:, bo:bo + bw])

    nc.compile()
    return nc


# ==================== host-side entry point ====================

_CACHED = {}


def _get_nc(N=N_FULL, NF=NF_FULL, CORES=CORES_FULL):
    key = (N, NF, CORES)
    if key not in _CACHED:
        _CACHED[key] = build_kernel(N, NF, CORES)
    return _CACHED[key]


def make_in_maps(inputs, N, CORES):
    import ml_dtypes

    NLOC = N // CORES
    NSUB = (N + 127) // 128
    bf = ml_dtypes.bfloat16
    f = {k: np.asarray(v, np.float64) for k, v in inputs.items()}
    LI = LAMBDA_INIT

    lam1 = np.exp(np.sum(f["lq1"] * f["lk1"]))
    lam2 = np.exp(np.sum(f["lq2"] * f["lk2"]))
    lam = lam1 - lam2 + LI
    mg, mb = f["mha_ln_g"], f["mha_ln_b"]
    Wk1 = f["k1_w"] * mg[:, None]; bk1 = mb @ f["k1_w"] + f["k1_b"]
    Wk2 = f["k2_w"] * mg[:, None]; bk2 = mb @ f["k2_w"] + f["k2_b"]
    Wv = f["v_w"] * mg[:, None]; bv = mb @ f["v_w"] + f["v_b"]
    Wq1 = f["q1_w"] * mg[:, None]; bq1 = mb @ f["q1_w"] + f["q1_b"]
    Wq2 = f["q2_w"] * mg[:, None]; bq2 = mb @ f["q2_w"] + f["q2_b"]
    Wob = f["attn_ln_g"][:, None] * f["out_w"] * (1 - LI)
    bo = (1 - LI) * (f["attn_ln_b"] @ f["out_w"]) + f["out_b"]
    W1p = f["ffn_ln_g"][:, None] * f["ffn1_w"]
    b1p = f["ffn_ln_b"] @ f["ffn1_w"] + f["ffn1_b"]

    kk = np.arange(1, 11)
    ang = f["e"][:, None] * kk / np.pi
    ne = (np.cos(ang) @ f["kan_a"] + np.sin(ang) @ f["kan_b"]
          + f["kan_bias"][0]) * f["alpha_w"][0, 0]
    ne_pad = np.zeros(NSUB * 128)
    ne_pad[:N] = ne
    ne_pm = np.ascontiguousarray(
        ne_pad.reshape(NSUB, 128).T.astype(np.float32))

    wb = np.concatenate(
        [f["fe_w1"].reshape(4, 128, 128)[i] for i in range(4)]
        + [f["fe_w2"], Wk1, Wk2, Wv, Wq1.T, -lam * Wq2.T, Wob, W1p,
           f["ffn2_w"]], axis=1)
    wb = np.ascontiguousarray(wb.astype(bf))
    colw = np.stack([f["fe_b1"], f["fe_b2"], bo, b1p, f["ffn2_b"],
                     np.zeros(128), np.zeros(128), np.zeros(128)], axis=1)
    colw = np.ascontiguousarray(colw.astype(np.float32))
    colb = np.ascontiguousarray(
        np.stack([bq1, -lam * bq2], axis=1).astype(bf))
    rowb = np.ascontiguousarray(
        np.concatenate([bk1, bk2, bv, np.zeros(128)])[None, :].astype(bf))
    roww = np.ascontiguousarray((N * bv)[None, :].astype(np.float32))

    x = np.asarray(inputs["x"], np.float32)
    u = np.asarray(inputs["u"], np.float32)
    in_maps = []
    for c in range(CORES):
        sh = u[c * NLOC:(c + 1) * NLOC]
        m = {
            "u": np.ascontiguousarray(sh.astype(bf)),
            "uT": np.ascontiguousarray(sh.T.astype(bf)),
            "xT": np.ascontiguousarray(
                x[c * NLOC:(c + 1) * NLOC].T.astype(bf)),
            "wb": wb, "colw": colw, "colb": colb, "rowb": rowb,
            "roww": roww, "ne": ne_pm,
        }
        in_maps.append(m)
    return in_maps


def assemble_out(res, CORES=CORES_FULL):
    # per-core outputs are [D, NLOC] (transposed); transpose + concat rows
    return np.concatenate(
        [np.asarray(res.results[c]["out"]).T for c in range(CORES)],
        axis=0).astype(np.float32)


def kernel(**inputs):
    from concourse import bass_utils

    nc = _get_nc()
    in_maps = make_in_maps(inputs, N_FULL, CORES_FULL)
    res = bass_utils.run_bass_kernel_spmd(nc, in_maps,
                                          core_ids=list(range(CORES_FULL)))
    return assemble_out(res)


if __name__ == "__main__":
    build_kernel()
    print("build ok")


# revision 44
# speedup vs baseline: 1.0565x; 1.0565x over previous
"""Trainium2 Bass kernel for nn_NoFoDifformer_FourierKAN (8-core SPMD), v4.

Sharding: u and nodes row-wise across 8 cores (1250 rows each). The z = u^T h
partial sums are all-reduced per column-chunk (bf16); the [d,d] Gram matrix of
the normalized activations rides inside the first chunk's all-reduce. Small
weights are pre-folded and packed on the HOST (LayerNorm affines folded into
projections, lambda and the FourierKAN filter evaluated in numpy) so the
device preamble is a handful of panel DMAs. Per-core outputs are [d, n_loc]
(transposed) and transposed+concatenated on the host.

Scheduling notes (engine FIFOs are in-order; a blocked DMA trigger blocks
everything behind it on that queue, so queue assignment is load-bearing):
- sync queue: xT, all u quarter-tiles, then all uT tiles, zc readbacks, out
  blocks. uT triggers drain only after the u triggers, so uT prefetch can
  never race the pass1-feeding u stream.
- scalar queue: activations + pass1 PSUM->staging casts (no uT triggers ->
  the LN Sqrt chain can never deadlock against uT pool recycling).
- gpsimd queue: weight panels, p1 staging writes, AR triggers. AR triggers
  serialize on the previous AR's completion, keeping the chain dense.
- ASCENDING chunk plan [2048, 4096, 3856]: the small chunk 0 puts the first
  AllReduce right at the end of the startup barrier (the AR chain is the
  critical path: ~20 GB/s alg ring bandwidth); the big middle ARs hide
  pass2/uT streaming; lnT+gram are emitted inside pass1(0) so the Gram
  rides chunk 0's AR with minimal doorbell delay.
- attention epilogue (watt/sT/haT) is emitted after the pass1 stages and
  accumulates commutatively into haT (initialized as a copy of hT).
"""

import numpy as np

N_FULL = 10000
NF_FULL = 512
D = 128
CORES_FULL = 8
LAMBDA_INIT = 0.2
GEXT = 256  # extra AR columns on chunk 0 for (G | s)
CH_PLAN = [2048, 4096, 3856]


def _splits(total, step):
    return [(o, min(step, total - o)) for o in range(0, total, step)]


def build_kernel(N=N_FULL, NF=NF_FULL, CORES=CORES_FULL):
    import concourse.bacc as bacc
    import concourse.tile as tile
    from concourse import mybir
    from concourse.masks import make_identity
    from contextlib import ExitStack

    dt = mybir.dt
    f32 = dt.float32
    bf16 = dt.bfloat16
    AF = mybir.ActivationFunctionType
    ALU = mybir.AluOpType

    NLOC = N // CORES                   # 1250
    ROWS = _splits(NLOC, 128)           # 9x128 + 98
    NT = len(ROWS)
    KX = NF // 128
    assert sum(CH_PLAN) == N
    assert all(w % 128 == 0 for w in CH_PLAN[:-1])
    CHUNKS = []
    off = 0
    for w in CH_PLAN:
        CHUNKS.append((off, w))
        off += w
    NCH = len(CHUNKS)
    CHMAX = max(CH_PLAN)
    NSUB = (N + 127) // 128             # 79
    BLK = _splits(NLOC, 512)            # [d, NLOC] op blocks
    rg = [list(range(CORES))]
    shared_space = "Shared" if CORES > 4 else "Local"

    nc = bacc.Bacc("TRN2", target_bir_lowering=False, debug=False,
                   num_devices=CORES)

    # ---------------- DRAM I/O ----------------
    t_u = nc.dram_tensor("u", [NLOC, N], bf16, kind="ExternalInput")
    t_uT = nc.dram_tensor("uT", [N, NLOC], bf16, kind="ExternalInput")
    t_xT = nc.dram_tensor("xT", [NF, NLOC], bf16, kind="ExternalInput")
    t_wb = nc.dram_tensor("wb", [128, 13 * 128], bf16, kind="ExternalInput")
    t_colw = nc.dram_tensor("colw", [128, 8], f32, kind="ExternalInput")
    t_colb = nc.dram_tensor("colb", [128, 2], bf16, kind="ExternalInput")
    t_rowb = nc.dram_tensor("rowb", [1, 4 * 128], bf16, kind="ExternalInput")
    t_roww = nc.dram_tensor("roww", [1, 128], f32, kind="ExternalInput")
    t_ne = nc.dram_tensor("ne", [128, NSUB], f32, kind="ExternalInput")
    t_out = nc.dram_tensor("out", [D, NLOC], f32, kind="ExternalOutput")

    with tile.TileContext(nc) as tc, ExitStack() as ctx:
        wpool = ctx.enter_context(tc.tile_pool(name="wpool", bufs=1))
        rowtmp = ctx.enter_context(tc.tile_pool(name="rowtmp", bufs=3))
        ustream = ctx.enter_context(tc.tile_pool(name="ustream", bufs=16))
        uTp = ctx.enter_context(tc.tile_pool(name="uTp", bufs=19))
        zcp = ctx.enter_context(tc.tile_pool(name="zcp", bufs=1))
        z16p = ctx.enter_context(tc.tile_pool(name="z16p", bufs=1))
        p1sbp = ctx.enter_context(tc.tile_pool(name="p1sbp", bufs=1))
        dram = ctx.enter_context(tc.tile_pool(name="dram", bufs=1, space="DRAM"))
        ps_p1 = ctx.enter_context(tc.tile_pool(name="ps_p1", bufs=2, space="PSUM"))
        ps_p2 = ctx.enter_context(tc.tile_pool(name="ps_p2", bufs=3, space="PSUM"))
        ps_mm = ctx.enter_context(tc.tile_pool(name="ps_mm", bufs=1, space="PSUM"))
        ps_t = ctx.enter_context(tc.tile_pool(name="ps_t", bufs=2, space="PSUM"))

        def p1_tile(w):
            return ps_p1.tile([128, 512], f32, tag="p1",
                              name=f"p1_{nc.next_id()}")[:, :w]

        def p2_tile(w):
            return ps_p2.tile([128, 512], f32, tag="p2",
                              name=f"p2_{nc.next_id()}")[:, :w]

        def mm_tile(p, w):
            return ps_mm.tile([128, 512], f32, tag="mmp",
                              name=f"mm_{nc.next_id()}")[:p, :w]

        def tb_tile(p, w):
            return ps_t.tile([128, 128], bf16, tag="pstb",
                             name=f"pstb_{nc.next_id()}")[:p, :w]

        def wtile(shape, dtype, name):
            return wpool.tile(shape, dtype, tag=name, name=name)

        def rtile(shape, dtype, tag):
            return rowtmp.tile(shape, dtype, tag=tag,
                               name=f"{tag}_{nc.next_id()}")

        def T(out_psum, in_sbuf, identity):
            nc.tensor.matmul(out_psum, in_sbuf, identity, is_transpose=True)

        # ================= constants & weights =================
        identb = wtile([128, 128], bf16, "identb")
        make_identity(nc, identb[:])
        ones_row_b = wtile([1, 128], bf16, "ones_row_b")
        nc.vector.memset(ones_row_b[:], 1.0)
        ones_col_b = wtile([128, 1], bf16, "ones_col_b")
        nc.vector.memset(ones_col_b[:], 1.0)
        oinv_col_b = wtile([128, 1], bf16, "oinv_col_b")
        nc.vector.memset(oinv_col_b[:], 1.0 / 128.0)
        eps_col = wtile([128, 1], f32, "eps_col")
        nc.vector.memset(eps_col[:], 1e-5)

        wb = wtile([128, 13 * 128], bf16, "wb")
        nc.gpsimd.dma_start(out=wb[:], in_=t_wb[:])
        colw = wtile([128, 8], f32, "colw")
        nc.gpsimd.dma_start(out=colw[:], in_=t_colw[:])
        colb = wtile([128, 2], bf16, "colb")
        nc.gpsimd.dma_start(out=colb[:], in_=t_colb[:])
        rowb = wtile([1, 4 * 128], bf16, "rowb")
        nc.gpsimd.dma_start(out=rowb[:], in_=t_rowb[:])
        roww = wtile([1, 128], f32, "roww")
        nc.gpsimd.dma_start(out=roww[:], in_=t_roww[:])
        ne = wtile([128, NSUB], f32, "ne")
        nc.gpsimd.dma_start(out=ne[:], in_=t_ne[:])

        def P(i):  # weight panel i of wb
            return wb[:, i * 128:(i + 1) * 128]
        few2b = P(4)
        Wk1b, Wk2b, Wvb = P(5), P(6), P(7)
        Wq1Tb, Wq2Tsb = P(8), P(9)
        Wob, W1pb, f2wb = P(10), P(11), P(12)
        feb1_c = colw[:, 0:1]
        feb2_c = colw[:, 1:2]
        bo_c = colw[:, 2:3]
        b1p_c = colw[:, 3:4]
        f2b_c = colw[:, 4:5]

        # xT into SBUF (4 partition k-tiles)
        xT4 = wtile([128, KX, NLOC], bf16, "xT4")
        for kt in range(KX):
            nc.sync.dma_start(out=xT4[:, kt, :],
                              in_=t_xT[kt * 128:(kt + 1) * 128, :])

        # ---------- u streaming loads (quarter tiles, sync queue) ----------
        u_tiles = {}

        def emit_u_loads(c):
            co, cw = CHUNKS[c]
            tiles = {}
            for q, (qo, qw) in enumerate(_splits(cw, 1024)):
                for r, (ro, rw) in enumerate(ROWS):
                    ut = ustream.tile([128, 1024], bf16, tag="u",
                                      name=f"u{c}_{q}_{r}")[:rw, :qw]
                    nc.sync.dma_start(
                        out=ut, in_=t_u[ro:ro + rw, co + qo:co + qo + qw])
                    tiles[(q, r)] = ut
            u_tiles[c] = tiles

        uT_tiles = {}

        def emit_uT_loads(c, eng=None):
            co, cw = CHUNKS[c]
            tl = []
            for t, (so, sw) in enumerate(_splits(cw, 128)):
                uTt = uTp.tile([128, NLOC], bf16, tag="uT",
                               name=f"uTl{c}_{t}")[:sw]
                (eng or nc.sync).dma_start(
                    out=uTt, in_=t_uT[co + so:co + so + sw, :])
                tl.append(uTt)
            uT_tiles[c] = tl

        for _c in range(NCH):
            emit_u_loads(_c)

        # ================= phase A: feature encoder (transposed) ==========
        hT = wtile([128, NLOC], f32, "hT")
        hTb = wtile([128, NLOC], bf16, "hTb")
        h16 = wtile([128, NT, D], bf16, "h16")
        for go, gw in BLK:
            psh1 = p2_tile(gw)
            for kt in range(KX):
                nc.tensor.matmul(psh1, P(kt), xT4[:, kt, go:go + gw],
                                 start=(kt == 0), stop=(kt == KX - 1))
            h1t = rtile([128, 512], bf16, "h1t")[:, :gw]
            nc.scalar.activation(h1t, psh1, AF.Relu, bias=feb1_c[:])
            pshT = p2_tile(gw)
            nc.tensor.matmul(pshT, few2b, h1t)
            nc.vector.tensor_scalar(hT[:, go:go + gw], pshT,
                                    scalar1=feb2_c, scalar2=None, op0=ALU.add)
            nc.scalar.activation(hTb[:, go:go + gw], hT[:, go:go + gw],
                                 AF.Copy)
        for r, (ro, rw) in enumerate(ROWS):
            pst = tb_tile(rw, 128)
            T(pst, hTb[:, ro:ro + rw], identb[:])
            nc.vector.tensor_copy(h16[:rw, r, :], pst)

        # ---------- transposed-layout LayerNorm helper ----------
        def lnT(x_sb, out_bf, pfx, xb=None):
            if xb is None:
                xb = wpool.tile([128, NLOC], bf16, tag="ln_xb",
                                name=f"{pfx}_xb")
                nc.scalar.activation(xb[:], x_sb[:], AF.Copy)
            x2b = wpool.tile([128, NLOC], bf16, tag="ln_x2b",
                             name=f"{pfx}_x2b")
            nc.vector.tensor_mul(x2b[:], x_sb[:], x_sb[:])

            def frow(tag, dt_):
                return rowtmp.tile([1, NLOC], dt_, tag=tag, bufs=2,
                                   name=f"{tag}_{nc.next_id()}")
            mrow = frow("ln_m", f32)
            rsrow = frow("ln_r", f32)
            for bo, bw in BLK:
                psm = mm_tile(1, bw)
                nc.tensor.matmul(psm, oinv_col_b[:], xb[:, bo:bo + bw])
                nc.vector.tensor_copy(mrow[:, bo:bo + bw], psm)
                psq_ = mm_tile(1, bw)
                nc.tensor.matmul(psq_, oinv_col_b[:], x2b[:, bo:bo + bw])
                nc.vector.tensor_mul(rsrow[:, bo:bo + bw],
                                     mrow[:, bo:bo + bw], mrow[:, bo:bo + bw])
                nc.vector.tensor_sub(rsrow[:, bo:bo + bw], psq_,
                                     rsrow[:, bo:bo + bw])         # var
            nc.scalar.activation(rsrow[:], rsrow[:], AF.Sqrt,
                                 bias=eps_col[:1])
            nc.vector.reciprocal(rsrow[:], rsrow[:])               # 1/sqrt
            m_b = frow("ln_mb", bf16)
            nc.vector.tensor_copy(m_b[:], mrow[:])
            rs_b = frow("ln_rb", bf16)
            nc.vector.tensor_copy(rs_b[:], rsrow[:])
            for bo, bw in BLK:
                psM = p2_tile(bw)
                nc.tensor.matmul(psM, ones_row_b[:], m_b[:, bo:bo + bw])
                psR = p2_tile(bw)
                nc.tensor.matmul(psR, ones_row_b[:], rs_b[:, bo:bo + bw])
                dtmp = rowtmp.tile([128, 512], f32, tag="btmp", bufs=2,
                                   name=f"lnd_{nc.next_id()}")[:, :bw]
                nc.vector.tensor_sub(dtmp, x_sb[:, bo:bo + bw], psM)
                nc.vector.tensor_mul(out_bf[:, bo:bo + bw], dtmp, psR)

        hnTb = wtile([128, NLOC], bf16, "hnTb")
        gram_sb = wtile([128, GEXT], bf16, "gram_sb")

        def emit_lngram():
            # LN(h) + Gram: G = hn^T hn, s = hn^T 1 (into gram_sb bf16)
            lnT(hT, hnTb, "hn", xb=hTb)
            psGS = ps_mm.tile([128, 512], f32, tag="mmp", name="psGS")
            for r, (ro, rw) in enumerate(ROWS):
                pst = tb_tile(rw, 128)
                T(pst, hnTb[:, ro:ro + rw], identb[:])
                hn_r = rtile([128, 128], bf16, "hn_r")[:rw]
                nc.vector.tensor_copy(hn_r, pst)
                nc.tensor.matmul(psGS[:, 0:128], hn_r, hn_r,
                                 start=(r == 0), stop=(r == NT - 1))
                nc.tensor.matmul(psGS[:1, 128:256], ones_col_b[:rw], hn_r,
                                 start=(r == 0), stop=(r == NT - 1))
            nc.vector.tensor_copy(gram_sb[:, 0:128], psGS[:, 0:128])
            nc.vector.tensor_copy(gram_sb[:1, 128:256], psGS[:1, 128:256])

        # ---------- DRAM staging ----------
        p1_in, p1_out = [], []
        for c, (co, cw) in enumerate(CHUNKS):
            w = cw + (GEXT if c == 0 else 0)
            p1_in.append(dram.tile([128, w], bf16, tag=f"p1in{c}",
                                   name=f"p1in{c}"))
            p1_out.append(dram.tile([128, w], bf16, tag=f"p1out{c}",
                                    name=f"p1out{c}", addr_space=shared_space))

        haT = wtile([128, NLOC], f32, "haT")
        sT = wtile([128, NLOC], f32, "sT")
        sTb = wpool.tile([128, NLOC], bf16, tag="hTb", name="sTb")
        aTb = wpool.tile([128, NLOC], bf16, tag="hnTb", name="aTb")

        def emit_pass1(c, mid=None):
            co, cw = CHUNKS[c]
            ut = u_tiles.pop(c)
            w = cw + (GEXT if c == 0 else 0)
            p1sb = p1sbp.tile([128, CHMAX + GEXT], bf16, tag="p1sb",
                              name=f"p1sb{c}")[:, :w]
            for q, (qo, qw) in enumerate(_splits(cw, 1024)):
                blocks = _splits(qw, 512)
                ps1 = [p1_tile(bw) for _, bw in blocks]
                for r, (ro, rw) in enumerate(ROWS):
                    for bi, (bo, bw) in enumerate(blocks):
                        nc.tensor.matmul(ps1[bi], h16[:rw, r, :],
                                         ut[(q, r)][:rw, bo:bo + bw],
                                         start=(r == 0), stop=(r == NT - 1))
                if q == 0 and mid is not None:
                    mid()
                for bi, (bo, bw) in enumerate(blocks):
                    nc.scalar.activation(p1sb[:, qo + bo:qo + bo + bw],
                                         ps1[bi], AF.Copy)
            if c == 0:
                nc.scalar.activation(p1sb[:, cw:cw + GEXT], gram_sb[:],
                                     AF.Copy)
            nc.gpsimd.dma_start(out=p1_in[c][:], in_=p1sb)
            nc.gpsimd.collective_compute(
                "AllReduce", ALU.add, replica_groups=rg,
                ins=[p1_in[c].opt()], outs=[p1_out[c].opt()])

        def emit_pass2(c):
            co, cw = CHUNKS[c]
            subs = _splits(cw, 128)
            zc = zcp.tile([128, CHMAX], bf16, tag="zc",
                          name=f"zc_{c}")[:, :cw]
            nc.scalar.dma_start(out=zc, in_=p1_out[c][:, :cw])
            z16 = z16p.tile([128, (CHMAX + 127) // 128, D], bf16, tag="z16",
                            name=f"z16_{c}")
            for t, (so, sw) in enumerate(subs):
                psz = tb_tile(sw, 128)
                T(psz, zc[:, so:so + sw], identb[:])
                gidx = (co + so) // 128
                nc.vector.tensor_scalar(z16[:sw, t, :], psz,
                                        scalar1=ne[:sw, gidx:gidx + 1],
                                        scalar2=None, op0=ALU.mult)
            uTc = uT_tiles.pop(c)
            ps2 = [p2_tile(iw) for _, iw in BLK]
            for t, (so, sw) in enumerate(subs):
                for ib, (io, iw) in enumerate(BLK):
                    nc.tensor.matmul(ps2[ib], z16[:sw, t, :],
                                     uTc[t][:sw, io:io + iw],
                                     start=(t == 0), stop=(t == len(subs) - 1))
            for ib, (io, iw) in enumerate(BLK):
                nc.vector.tensor_add(haT[:, io:io + iw],
                                     haT[:, io:io + iw], ps2[ib])

        def emit_att():
            # post-AR0 attention path: gram -> k1v/k2v -> Watt -> sT -> aT -> haT
            co0, cw0 = CHUNKS[0]
            gkv = wtile([128, GEXT], bf16, "gkv")
            nc.scalar.dma_start(out=gkv[:], in_=p1_out[0][:, cw0:cw0 + GEXT])
            G_b = gkv[:, 0:128]
            s_row = gkv[:1, 128:256]
            psc = tb_tile(128, 1)
            T(psc, s_row, identb[:1, :1])
            s_col = rtile([128, 1], bf16, "s_col")
            nc.vector.tensor_copy(s_col[:], psc)
            # X1 = G Wv + s (.) bv   (shared by k1v and k2v)
            psX = mm_tile(128, 128)
            nc.tensor.matmul(psX, G_b, Wvb, start=True, stop=False)
            nc.tensor.matmul(psX, s_row, rowb[:1, 256:384], start=False,
                             stop=True)
            X1b = wtile([128, 128], bf16, "X1b")
            nc.vector.tensor_copy(X1b[:], psX)
            # rrow = s^T Wv + N bv
            psr = mm_tile(1, 128)
            nc.tensor.matmul(psr, s_col[:], Wvb)
            rrow = rtile([1, 128], f32, "rrow")
            nc.vector.tensor_add(rrow[:], psr, roww[:1])
            rrow_b = rtile([1, 128], bf16, "rrow_b")
            nc.vector.tensor_copy(rrow_b[:], rrow[:])
            kvs = []
            for i, Wk in ((0, Wk1b), (1, Wk2b)):
                psK = mm_tile(128, 128)
                nc.tensor.matmul(psK, Wk, X1b[:], start=True, stop=False)
                nc.tensor.matmul(psK, rowb[:1, i * 128:(i + 1) * 128],
                                 rrow_b[:], start=False, stop=True)
                kv = wtile([128, 128], bf16, f"k{i+1}v_b")
                nc.vector.tensor_copy(kv[:], psK)
                kvs.append(kv)
            psW = mm_tile(128, 128)
            nc.tensor.matmul(psW, Wq1Tb, kvs[0][:], start=True, stop=False)
            nc.tensor.matmul(psW, Wq2Tsb, kvs[1][:], start=False, stop=True)
            Wattb = wtile([128, D], bf16, "Wattb")
            nc.vector.tensor_copy(Wattb[:], psW)
            psB = mm_tile(128, 1)
            nc.tensor.matmul(psB, kvs[0][:], colb[:, 0:1], start=True,
                             stop=False)
            nc.tensor.matmul(psB, kvs[1][:], colb[:, 1:2], start=False,
                             stop=True)
            batt_c = wtile([128, 1], f32, "batt_c")
            nc.vector.tensor_copy(batt_c[:], psB)
            # sT = Watt^T @ hnT + batt  (transposed layout)
            for bo, bw in BLK:
                pss = p2_tile(bw)
                nc.tensor.matmul(pss, Wattb[:], hnTb[:, bo:bo + bw])
                nc.vector.tensor_scalar(sT[:, bo:bo + bw], pss,
                                        scalar1=batt_c[:], scalar2=None,
                                        op0=ALU.add)
                nc.vector.tensor_copy(sTb[:, bo:bo + bw], sT[:, bo:bo + bw])
            lnT(sT, aTb, "s", xb=sTb)
            # haT += Wo'^T @ aT + bo   (commutative accumulation)
            for bo, bw in BLK:
                psa = p2_tile(bw)
                nc.tensor.matmul(psa, Wob, aTb[:, bo:bo + bw])
                atmp = rowtmp.tile([128, 512], f32, tag="btmp", bufs=2,
                                   name=f"atmp_{nc.next_id()}")[:, :bw]
                nc.vector.tensor_scalar(atmp, psa, scalar1=bo_c,
                                        scalar2=None, op0=ALU.add)
                nc.vector.tensor_add(haT[:, bo:bo + bw],
                                     haT[:, bo:bo + bw], atmp)

        # ---- pipeline ----
        emit_pass1(0, mid=emit_lngram)
        for _c in range(1, NCH):
            emit_pass1(_c)
        for _c in range(NCH):
            emit_uT_loads(_c)
        # haT starts as a copy of hT; att and pass2 then += into it
        nc.vector.tensor_copy(haT[:], hT[:])
        emit_att()           # runs in the gram-AR -> AR0 gap (Tensor idle)
        for _c in range(NCH):
            emit_pass2(_c)

        # ======= final epilogue: fused per-block LN+FFN, early out DMA =====
        outT = sT  # sT is dead after lnT(sT); reuse its buffer
        fxb = wpool.tile([128, NLOC], bf16, tag="hTb", name="fxb")
        fx2 = wpool.tile([128, NLOC], bf16, tag="ln_x2b", name="fx2")
        mrow = rowtmp.tile([1, NLOC], f32, tag="ep_m", bufs=1, name="ep_m")
        vrow = rowtmp.tile([1, NLOC], f32, tag="ep_v", bufs=1, name="ep_v")
        for bo, bw in BLK:
            nc.scalar.activation(fxb[:, bo:bo + bw], haT[:, bo:bo + bw],
                                 AF.Copy)
            nc.vector.tensor_mul(fx2[:, bo:bo + bw], haT[:, bo:bo + bw],
                                 haT[:, bo:bo + bw])
            psm = mm_tile(1, bw)
            nc.tensor.matmul(psm, oinv_col_b[:], fxb[:, bo:bo + bw])
            nc.vector.tensor_copy(mrow[:, bo:bo + bw], psm)
            psq_ = mm_tile(1, bw)
            nc.tensor.matmul(psq_, oinv_col_b[:], fx2[:, bo:bo + bw])
            nc.vector.tensor_mul(vrow[:, bo:bo + bw], mrow[:, bo:bo + bw],
                                 mrow[:, bo:bo + bw])
            nc.vector.tensor_sub(vrow[:, bo:bo + bw], psq_,
                                 vrow[:, bo:bo + bw])
        nc.scalar.activation(vrow[:], vrow[:], AF.Sqrt, bias=eps_col[:1])
        nc.vector.reciprocal(vrow[:], vrow[:])
        m_bf = rowtmp.tile([1, NLOC], bf16, tag="ep_mb", bufs=1, name="ep_mb")
        nc.vector.tensor_copy(m_bf[:], mrow[:])
        rs_bf = rowtmp.tile([1, NLOC], bf16, tag="ep_rb", bufs=1,
                            name="ep_rb")
        nc.vector.tensor_copy(rs_bf[:], vrow[:])
        stats = [(m_bf[:, bo:bo + bw], rs_bf[:, bo:bo + bw])
                 for bo, bw in BLK]
        for (bo, bw), (m_b, rs_b) in zip(BLK, stats):
            psM = p2_tile(bw)
            nc.tensor.matmul(psM, ones_row_b[:], m_b)
            psR = p2_tile(bw)
            nc.tensor.matmul(psR, ones_row_b[:], rs_b)
            dtmp = rowtmp.tile([128, 512], f32, tag="btmp", bufs=2,
                               name=f"lnd_{nc.next_id()}")[:, :bw]
            nc.vector.tensor_sub(dtmp, haT[:, bo:bo + bw], psM)
            fb_ = rtile([128, 512], bf16, "fb")[:, :bw]
            nc.vector.tensor_mul(fb_, dtmp, psR)
            psg_ = p2_tile(bw)
            nc.tensor.matmul(psg_, W1pb, fb_)
            gb_ = rtile([128, 512], bf16, "gb")[:, :bw]
            nc.scalar.activation(gb_, psg_, AF.Gelu, bias=b1p_c[:])
            pso_ = mm_tile(128, bw)
            nc.tensor.matmul(pso_, f2wb, gb_)
            otmp = rowtmp.tile([128, 512], f32, tag="btmp", bufs=2,
                               name=f"otmp_{nc.next_id()}")[:, :bw]
            nc.vector.tensor_scalar(otmp, pso_, scalar1=f2b_c,
                                    scalar2=None, op0=ALU.add)
            nc.vector.tensor_add(outT[:, bo:bo + bw],
                                 haT[:, bo:bo + bw], otmp)
            nc.sync.dma_start(out=t_out[:, bo:bo + bw],
                              in_=outT[:, bo:bo + bw])

    nc.compile()
    return nc


# ==================== host-side entry point ====================

_CACHED = {}


def _get_nc(N=N_FULL, NF=NF_FULL, CORES=CORES_FULL):
    key = (N, NF, CORES)
    if key not in _CACHED:
        _CACHED[key] = build_kernel(N, NF, CORES)
    return _CACHED[key]


def make_in_maps(inputs, N, CORES):
    import ml_dtypes

    NLOC = N // CORES
    NSUB = (N + 127) // 128
    bf = ml_dtypes.bfloat16
    f = {k: np.asarray(v, np.float64) for k, v in inputs.items()}
    LI = LAMBDA_INIT

    lam1 = np.exp(np.sum(f["lq1"] * f["lk1"]))
    lam2 = np.exp(np.sum(f["lq2"] * f["lk2"]))
    lam = lam1 - lam2 + LI
    mg, mb = f["mha_ln_g"], f["mha_ln_b"]
    Wk1 = f["k1_w"] * mg[:, None]; bk1 = mb @ f["k1_w"] + f["k1_b"]
    Wk2 = f["k2_w"] * mg[:, None]; bk2 = mb @ f["k2_w"] + f["k2_b"]
    Wv = f["v_w"] * mg[:, None]; bv = mb @ f["v_w"] + f["v_b"]
    Wq1 = f["q1_w"] * mg[:, None]; bq1 = mb @ f["q1_w"] + f["q1_b"]
    Wq2 = f["q2_w"] * mg[:, None]; bq2 = mb @ f["q2_w"] + f["q2_b"]
    Wob = f["attn_ln_g"][:, None] * f["out_w"] * (1 - LI)
    bo = (1 - LI) * (f["attn_ln_b"] @ f["out_w"]) + f["out_b"]
    W1p = f["ffn_ln_g"][:, None] * f["ffn1_w"]
    b1p = f["ffn_ln_b"] @ f["ffn1_w"] + f["ffn1_b"]

    kk = np.arange(1, 11)
    ang = f["e"][:, None] * kk / np.pi
    ne = (np.cos(ang) @ f["kan_a"] + np.sin(ang) @ f["kan_b"]
          + f["kan_bias"][0]) * f["alpha_w"][0, 0]
    ne_pad = np.zeros(NSUB * 128)
    ne_pad[:N] = ne
    ne_pm = np.ascontiguousarray(
        ne_pad.reshape(NSUB, 128).T.astype(np.float32))

    wb = np.concatenate(
        [f["fe_w1"].reshape(4, 128, 128)[i] for i in range(4)]
        + [f["fe_w2"], Wk1, Wk2, Wv, Wq1.T, -lam * Wq2.T, Wob, W1p,
           f["ffn2_w"]], axis=1)
    wb = np.ascontiguousarray(wb.astype(bf))
    colw = np.stack([f["fe_b1"], f["fe_b2"], bo, b1p, f["ffn2_b"],
                     np.zeros(128), np.zeros(128), np.zeros(128)], axis=1)
    colw = np.ascontiguousarray(colw.astype(np.float32))
    colb = np.ascontiguousarray(
        np.stack([bq1, -lam * bq2], axis=1).astype(bf))
    rowb = np.ascontiguousarray(
        np.concatenate([bk1, bk2, bv, np.zeros(128)])[None, :].astype(bf))
    roww = np.ascontiguousarray((N * bv)[None, :].astype(np.float32))

    x = np.asarray(inputs["x"], np.float32)
    u = np.asarray(inputs["u"], np.float32)
    in_maps = []
    for c in range(CORES):
        sh = u[c * NLOC:(c + 1) * NLOC]
        m = {
            "u": np.ascontiguousarray(sh.astype(bf)),
            "uT": np.ascontiguousarray(sh.T.astype(bf)),
            "xT": np.ascontiguousarray(
                x[c * NLOC:(c + 1) * NLOC].T.astype(bf)),
            "wb": wb, "colw": colw, "colb": colb, "rowb": rowb,
            "roww": roww, "ne": ne_pm,
        }
        in_maps.append(m)
    return in_maps


def assemble_out(res, CORES=CORES_FULL):
    # per-core outputs are [D, NLOC] (transposed); transpose + concat rows
    return np.concatenate(
        [np.asarray(res.results[c]["out"]).T for c in range(CORES)],
        axis=0).astype(np.float32)


def kernel(**inputs):
    from concourse import bass_utils

    nc = _get_nc()
    in_maps = make_in_maps(inputs, N_FULL, CORES_FULL)
    res = bass_utils.run_bass_kernel_spmd(nc, in_maps,
                                          core_ids=list(range(CORES_FULL)))
    return assemble_out(res)


if __name__ == "__main__":
    build_kernel()
    print("build ok")


# revision 45
# speedup vs baseline: 1.1060x; 1.0469x over previous
"""Trainium2 Bass kernel for nn_NoFoDifformer_FourierKAN (8-core SPMD), v4.

Sharding: u and nodes row-wise across 8 cores (1250 rows each). The z = u^T h
partial sums are all-reduced per column-chunk (bf16); the [d,d] Gram matrix of
the normalized activations rides inside the first chunk's all-reduce. Small
weights are pre-folded and packed on the HOST (LayerNorm affines folded into
projections, lambda and the FourierKAN filter evaluated in numpy) so the
device preamble is a handful of panel DMAs. Per-core outputs are [d, n_loc]
(transposed) and transposed+concatenated on the host.

Scheduling notes (engine FIFOs are in-order; a blocked DMA trigger blocks
everything behind it on that queue, so queue assignment is load-bearing):
- sync queue: xT, all u quarter-tiles, then all uT tiles, zc readbacks, out
  blocks. uT triggers drain only after the u triggers, so uT prefetch can
  never race the pass1-feeding u stream.
- scalar queue: activations + pass1 PSUM->staging casts (no uT triggers ->
  the LN Sqrt chain can never deadlock against uT pool recycling).
- gpsimd queue: weight panels, p1 staging writes, AR triggers. AR triggers
  serialize on the previous AR's completion, keeping the chain dense.
- ASCENDING chunk plan [2048, 4096, 3856]: the small chunk 0 puts the first
  AllReduce right at the end of the startup barrier (the AR chain is the
  critical path: ~20 GB/s alg ring bandwidth); the big middle ARs hide
  pass2/uT streaming; lnT+gram are emitted inside pass1(0) so the Gram
  rides chunk 0's AR with minimal doorbell delay.
- attention epilogue (watt/sT/haT) is emitted after the pass1 stages and
  accumulates commutatively into haT (initialized as a copy of hT).
"""

import numpy as np

N_FULL = 10000
NF_FULL = 512
D = 128
CORES_FULL = 8
LAMBDA_INIT = 0.2
GEXT = 256  # extra AR columns on chunk 0 for (G | s)
CH_PLAN = [2048, 4096, 3856]


def _splits(total, step):
    return [(o, min(step, total - o)) for o in range(0, total, step)]


def build_kernel(N=N_FULL, NF=NF_FULL, CORES=CORES_FULL):
    import concourse.bacc as bacc
    import concourse.tile as tile
    from concourse import mybir
    from concourse.masks import make_identity
    from contextlib import ExitStack

    dt = mybir.dt
    f32 = dt.float32
    bf16 = dt.bfloat16
    AF = mybir.ActivationFunctionType
    ALU = mybir.AluOpType

    NLOC = N // CORES                   # 1250
    ROWS = _splits(NLOC, 128)           # 9x128 + 98
    NT = len(ROWS)
    KX = NF // 128
    assert sum(CH_PLAN) == N
    assert all(w % 128 == 0 for w in CH_PLAN[:-1])
    CHUNKS = []
    off = 0
    for w in CH_PLAN:
        CHUNKS.append((off, w))
        off += w
    NCH = len(CHUNKS)
    CHMAX = max(CH_PLAN)
    NSUB = (N + 127) // 128             # 79
    BLK = _splits(NLOC, 512)            # [d, NLOC] op blocks
    rg = [list(range(CORES))]
    shared_space = "Shared" if CORES > 4 else "Local"

    nc = bacc.Bacc("TRN2", target_bir_lowering=False, debug=False,
                   num_devices=CORES)

    # ---------------- DRAM I/O ----------------
    t_u = nc.dram_tensor("u", [NLOC, N], bf16, kind="ExternalInput")
    t_uT = nc.dram_tensor("uT", [N, NLOC], bf16, kind="ExternalInput")
    t_xT = nc.dram_tensor("xT", [NF, NLOC], bf16, kind="ExternalInput")
    t_wb = nc.dram_tensor("wb", [128, 13 * 128], bf16, kind="ExternalInput")
    t_colw = nc.dram_tensor("colw", [128, 8], f32, kind="ExternalInput")
    t_colb = nc.dram_tensor("colb", [128, 2], bf16, kind="ExternalInput")
    t_rowb = nc.dram_tensor("rowb", [1, 4 * 128], bf16, kind="ExternalInput")
    t_roww = nc.dram_tensor("roww", [1, 128], f32, kind="ExternalInput")
    t_ne = nc.dram_tensor("ne", [128, NSUB], f32, kind="ExternalInput")
    t_out = nc.dram_tensor("out", [D, NLOC], f32, kind="ExternalOutput")

    with tile.TileContext(nc) as tc, ExitStack() as ctx:
        wpool = ctx.enter_context(tc.tile_pool(name="wpool", bufs=1))
        rowtmp = ctx.enter_context(tc.tile_pool(name="rowtmp", bufs=3))
        ustream = ctx.enter_context(tc.tile_pool(name="ustream", bufs=16))
        uTp = ctx.enter_context(tc.tile_pool(name="uTp", bufs=19))
        zcp = ctx.enter_context(tc.tile_pool(name="zcp", bufs=1))
        z16p = ctx.enter_context(tc.tile_pool(name="z16p", bufs=1))
        p1sbp = ctx.enter_context(tc.tile_pool(name="p1sbp", bufs=1))
        dram = ctx.enter_context(tc.tile_pool(name="dram", bufs=1, space="DRAM"))
        ps_p1 = ctx.enter_context(tc.tile_pool(name="ps_p1", bufs=2, space="PSUM"))
        ps_p2 = ctx.enter_context(tc.tile_pool(name="ps_p2", bufs=3, space="PSUM"))
        ps_mm = ctx.enter_context(tc.tile_pool(name="ps_mm", bufs=1, space="PSUM"))
        ps_t = ctx.enter_context(tc.tile_pool(name="ps_t", bufs=2, space="PSUM"))

        def p1_tile(w):
            return ps_p1.tile([128, 512], f32, tag="p1",
                              name=f"p1_{nc.next_id()}")[:, :w]

        def p2_tile(w):
            return ps_p2.tile([128, 512], f32, tag="p2",
                              name=f"p2_{nc.next_id()}")[:, :w]

        def mm_tile(p, w):
            return ps_mm.tile([128, 512], f32, tag="mmp",
                              name=f"mm_{nc.next_id()}")[:p, :w]

        def tb_tile(p, w):
            return ps_t.tile([128, 128], bf16, tag="pstb",
                             name=f"pstb_{nc.next_id()}")[:p, :w]

        def wtile(shape, dtype, name):
            return wpool.tile(shape, dtype, tag=name, name=name)

        def rtile(shape, dtype, tag):
            return rowtmp.tile(shape, dtype, tag=tag,
                               name=f"{tag}_{nc.next_id()}")

        def T(out_psum, in_sbuf, identity):
            nc.tensor.matmul(out_psum, in_sbuf, identity, is_transpose=True)

        # ================= constants & weights =================
        identb = wtile([128, 128], bf16, "identb")
        make_identity(nc, identb[:])
        ones_row_b = wtile([1, 128], bf16, "ones_row_b")
        nc.vector.memset(ones_row_b[:], 1.0)
        ones_col_b = wtile([128, 1], bf16, "ones_col_b")
        nc.vector.memset(ones_col_b[:], 1.0)
        oinv_col_b = wtile([128, 1], bf16, "oinv_col_b")
        nc.vector.memset(oinv_col_b[:], 1.0 / 128.0)
        eps_col = wtile([128, 1], f32, "eps_col")
        nc.vector.memset(eps_col[:], 1e-5)

        wb = wtile([128, 13 * 128], bf16, "wb")
        nc.gpsimd.dma_start(out=wb[:], in_=t_wb[:])
        colw = wtile([128, 8], f32, "colw")
        nc.gpsimd.dma_start(out=colw[:], in_=t_colw[:])
        colb = wtile([128, 2], bf16, "colb")
        nc.gpsimd.dma_start(out=colb[:], in_=t_colb[:])
        rowb = wtile([1, 4 * 128], bf16, "rowb")
        nc.gpsimd.dma_start(out=rowb[:], in_=t_rowb[:])
        roww = wtile([1, 128], f32, "roww")
        nc.gpsimd.dma_start(out=roww[:], in_=t_roww[:])
        ne = wtile([128, NSUB], f32, "ne")
        nc.gpsimd.dma_start(out=ne[:], in_=t_ne[:])

        def P(i):  # weight panel i of wb
            return wb[:, i * 128:(i + 1) * 128]
        few2b = P(4)
        Wk1b, Wk2b, Wvb = P(5), P(6), P(7)
        Wq1Tb, Wq2Tsb = P(8), P(9)
        Wob, W1pb, f2wb = P(10), P(11), P(12)
        feb1_c = colw[:, 0:1]
        feb2_c = colw[:, 1:2]
        bo_c = colw[:, 2:3]
        b1p_c = colw[:, 3:4]
        f2b_c = colw[:, 4:5]

        # xT into SBUF (4 partition k-tiles)
        xT4 = wtile([128, KX, NLOC], bf16, "xT4")
        for kt in range(KX):
            nc.sync.dma_start(out=xT4[:, kt, :],
                              in_=t_xT[kt * 128:(kt + 1) * 128, :])

        # ---------- u streaming loads (quarter tiles, sync queue) ----------
        u_tiles = {}

        def emit_u_loads(c):
            co, cw = CHUNKS[c]
            tiles = {}
            for q, (qo, qw) in enumerate(_splits(cw, 1024)):
                for r, (ro, rw) in enumerate(ROWS):
                    ut = ustream.tile([128, 1024], bf16, tag="u",
                                      name=f"u{c}_{q}_{r}")[:rw, :qw]
                    nc.sync.dma_start(
                        out=ut, in_=t_u[ro:ro + rw, co + qo:co + qo + qw])
                    tiles[(q, r)] = ut
            u_tiles[c] = tiles

        uT_tiles = {}

        def emit_uT_loads(c, eng=None):
            co, cw = CHUNKS[c]
            tl = []
            for t, (so, sw) in enumerate(_splits(cw, 128)):
                uTt = uTp.tile([128, NLOC], bf16, tag="uT",
                               name=f"uTl{c}_{t}")[:sw]
                (eng or nc.sync).dma_start(
                    out=uTt, in_=t_uT[co + so:co + so + sw, :])
                tl.append(uTt)
            uT_tiles[c] = tl

        for _c in range(NCH):
            emit_u_loads(_c)

        # ================= phase A: feature encoder (transposed) ==========
        hT = wtile([128, NLOC], f32, "hT")
        hTb = wtile([128, NLOC], bf16, "hTb")
        h16 = wtile([128, NT, D], bf16, "h16")
        for go, gw in BLK:
            psh1 = p2_tile(gw)
            for kt in range(KX):
                nc.tensor.matmul(psh1, P(kt), xT4[:, kt, go:go + gw],
                                 start=(kt == 0), stop=(kt == KX - 1))
            h1t = rtile([128, 512], bf16, "h1t")[:, :gw]
            nc.scalar.activation(h1t, psh1, AF.Relu, bias=feb1_c[:])
            pshT = p2_tile(gw)
            nc.tensor.matmul(pshT, few2b, h1t)
            nc.vector.tensor_scalar(hT[:, go:go + gw], pshT,
                                    scalar1=feb2_c, scalar2=None, op0=ALU.add)
            nc.scalar.activation(hTb[:, go:go + gw], hT[:, go:go + gw],
                                 AF.Copy)
        for r, (ro, rw) in enumerate(ROWS):
            pst = tb_tile(rw, 128)
            T(pst, hTb[:, ro:ro + rw], identb[:])
            nc.vector.tensor_copy(h16[:rw, r, :], pst)

        # ---------- transposed-layout LayerNorm helper ----------
        def lnT(x_sb, out_bf, pfx, xb=None):
            if xb is None:
                xb = wpool.tile([128, NLOC], bf16, tag="ln_xb",
                                name=f"{pfx}_xb")
                nc.scalar.activation(xb[:], x_sb[:], AF.Copy)
            x2b = wpool.tile([128, NLOC], bf16, tag="ln_x2b",
                             name=f"{pfx}_x2b")
            nc.vector.tensor_mul(x2b[:], x_sb[:], x_sb[:])

            def frow(tag, dt_):
                return rowtmp.tile([1, NLOC], dt_, tag=tag, bufs=2,
                                   name=f"{tag}_{nc.next_id()}")
            mrow = frow("ln_m", f32)
            rsrow = frow("ln_r", f32)
            for bo, bw in BLK:
                psm = mm_tile(1, bw)
                nc.tensor.matmul(psm, oinv_col_b[:], xb[:, bo:bo + bw])
                nc.vector.tensor_copy(mrow[:, bo:bo + bw], psm)
                psq_ = mm_tile(1, bw)
                nc.tensor.matmul(psq_, oinv_col_b[:], x2b[:, bo:bo + bw])
                nc.vector.tensor_mul(rsrow[:, bo:bo + bw],
                                     mrow[:, bo:bo + bw], mrow[:, bo:bo + bw])
                nc.vector.tensor_sub(rsrow[:, bo:bo + bw], psq_,
                                     rsrow[:, bo:bo + bw])         # var
            nc.scalar.activation(rsrow[:], rsrow[:], AF.Sqrt,
                                 bias=eps_col[:1])
            nc.vector.reciprocal(rsrow[:], rsrow[:])               # 1/sqrt
            m_b = frow("ln_mb", bf16)
            nc.vector.tensor_copy(m_b[:], mrow[:])
            rs_b = frow("ln_rb", bf16)
            nc.vector.tensor_copy(rs_b[:], rsrow[:])
            for bo, bw in BLK:
                psM = p2_tile(bw)
                nc.tensor.matmul(psM, ones_row_b[:], m_b[:, bo:bo + bw])
                psR = p2_tile(bw)
                nc.tensor.matmul(psR, ones_row_b[:], rs_b[:, bo:bo + bw])
                dtmp = rowtmp.tile([128, 512], f32, tag="btmp", bufs=2,
                                   name=f"lnd_{nc.next_id()}")[:, :bw]
                nc.vector.tensor_sub(dtmp, x_sb[:, bo:bo + bw], psM)
                nc.vector.tensor_mul(out_bf[:, bo:bo + bw], dtmp, psR)

        hnTb = wtile([128, NLOC], bf16, "hnTb")
        gram_sb = wtile([128, GEXT], bf16, "gram_sb")

        def emit_lngram():
            # Gram straight from h16 row tiles: per-row bn_stats LN (tiny
            # [rw,1] reciprocals, no transposes) so G = hn^T hn, s = hn^T 1
            # are ready ~30us earlier than via the full-width lnT. The
            # transposed hnTb for the attention path is produced later by
            # lnT(hT) off the AR0 critical path.
            psGS = ps_mm.tile([128, 512], f32, tag="mmp", name="psGS")
            for r, (ro, rw) in enumerate(ROWS):
                stats = rtile([128, 6], f32, "gst")
                nc.vector.bn_stats(stats[:rw], h16[:rw, r, :])
                mv = rtile([128, 2], f32, "gmv")
                nc.vector.bn_aggr(mv[:rw], stats[:rw])
                rs = rtile([128, 1], f32, "grs")
                nc.scalar.activation(rs[:rw], mv[:rw, 1:2], AF.Sqrt,
                                     bias=eps_col[:rw])
                nc.vector.reciprocal(rs[:rw], rs[:rw])
                hn_r = rtile([128, 128], bf16, "hn_r")[:rw]
                nc.vector.tensor_scalar(hn_r, h16[:rw, r, :],
                                        scalar1=mv[:rw, 0:1],
                                        op0=ALU.subtract,
                                        scalar2=rs[:rw], op1=ALU.mult)
                nc.tensor.matmul(psGS[:, 0:128], hn_r, hn_r,
                                 start=(r == 0), stop=(r == NT - 1))
                nc.tensor.matmul(psGS[:1, 128:256], ones_col_b[:rw], hn_r,
                                 start=(r == 0), stop=(r == NT - 1))
            nc.vector.tensor_copy(gram_sb[:, 0:128], psGS[:, 0:128])
            nc.vector.tensor_copy(gram_sb[:1, 128:256], psGS[:1, 128:256])

        # ---------- DRAM staging ----------
        p1_in, p1_out = [], []
        for c, (co, cw) in enumerate(CHUNKS):
            w = cw + (GEXT if c == 0 else 0)
            p1_in.append(dram.tile([128, w], bf16, tag=f"p1in{c}",
                                   name=f"p1in{c}"))
            p1_out.append(dram.tile([128, w], bf16, tag=f"p1out{c}",
                                    name=f"p1out{c}", addr_space=shared_space))

        haT = wtile([128, NLOC], f32, "haT")
        sT = wtile([128, NLOC], f32, "sT")
        sTb = wpool.tile([128, NLOC], bf16, tag="hTb", name="sTb")
        aTb = wpool.tile([128, NLOC], bf16, tag="hnTb", name="aTb")

        def emit_pass1(c, mid=None):
            co, cw = CHUNKS[c]
            ut = u_tiles.pop(c)
            w = cw + (GEXT if c == 0 else 0)
            p1sb = p1sbp.tile([128, CHMAX + GEXT], bf16, tag="p1sb",
                              name=f"p1sb{c}")[:, :w]
            for q, (qo, qw) in enumerate(_splits(cw, 1024)):
                blocks = _splits(qw, 512)
                ps1 = [p1_tile(bw) for _, bw in blocks]
                for r, (ro, rw) in enumerate(ROWS):
                    for bi, (bo, bw) in enumerate(blocks):
                        nc.tensor.matmul(ps1[bi], h16[:rw, r, :],
                                         ut[(q, r)][:rw, bo:bo + bw],
                                         start=(r == 0), stop=(r == NT - 1))
                if q == 0 and mid is not None:
                    mid()
                for bi, (bo, bw) in enumerate(blocks):
                    nc.scalar.activation(p1sb[:, qo + bo:qo + bo + bw],
                                         ps1[bi], AF.Copy)
            if c == 0:
                nc.scalar.activation(p1sb[:, cw:cw + GEXT], gram_sb[:],
                                     AF.Copy)
            nc.gpsimd.dma_start(out=p1_in[c][:], in_=p1sb)
            nc.gpsimd.collective_compute(
                "AllReduce", ALU.add, replica_groups=rg,
                ins=[p1_in[c].opt()], outs=[p1_out[c].opt()])

        def emit_pass2(c):
            co, cw = CHUNKS[c]
            subs = _splits(cw, 128)
            zc = zcp.tile([128, CHMAX], bf16, tag="zc",
                          name=f"zc_{c}")[:, :cw]
            nc.scalar.dma_start(out=zc, in_=p1_out[c][:, :cw])
            z16 = z16p.tile([128, (CHMAX + 127) // 128, D], bf16, tag="z16",
                            name=f"z16_{c}")
            for t, (so, sw) in enumerate(subs):
                psz = tb_tile(sw, 128)
                T(psz, zc[:, so:so + sw], identb[:])
                gidx = (co + so) // 128
                nc.vector.tensor_scalar(z16[:sw, t, :], psz,
                                        scalar1=ne[:sw, gidx:gidx + 1],
                                        scalar2=None, op0=ALU.mult)
            uTc = uT_tiles.pop(c)
            ps2 = [p2_tile(iw) for _, iw in BLK]
            for t, (so, sw) in enumerate(subs):
                for ib, (io, iw) in enumerate(BLK):
                    nc.tensor.matmul(ps2[ib], z16[:sw, t, :],
                                     uTc[t][:sw, io:io + iw],
                                     start=(t == 0), stop=(t == len(subs) - 1))
            for ib, (io, iw) in enumerate(BLK):
                nc.vector.tensor_add(haT[:, io:io + iw],
                                     haT[:, io:io + iw], ps2[ib])

        def emit_att():
            # post-AR0 attention path: gram -> k1v/k2v -> Watt -> sT -> aT -> haT
            co0, cw0 = CHUNKS[0]
            gkv = wtile([128, GEXT], bf16, "gkv")
            nc.scalar.dma_start(out=gkv[:], in_=p1_out[0][:, cw0:cw0 + GEXT])
            G_b = gkv[:, 0:128]
            s_row = gkv[:1, 128:256]
            psc = tb_tile(128, 1)
            T(psc, s_row, identb[:1, :1])
            s_col = rtile([128, 1], bf16, "s_col")
            nc.vector.tensor_copy(s_col[:], psc)
            # X1 = G Wv + s (.) bv   (shared by k1v and k2v)
            psX = mm_tile(128, 128)
            nc.tensor.matmul(psX, G_b, Wvb, start=True, stop=False)
            nc.tensor.matmul(psX, s_row, rowb[:1, 256:384], start=False,
                             stop=True)
            X1b = wtile([128, 128], bf16, "X1b")
            nc.vector.tensor_copy(X1b[:], psX)
            # rrow = s^T Wv + N bv
            psr = mm_tile(1, 128)
            nc.tensor.matmul(psr, s_col[:], Wvb)
            rrow = rtile([1, 128], f32, "rrow")
            nc.vector.tensor_add(rrow[:], psr, roww[:1])
            rrow_b = rtile([1, 128], bf16, "rrow_b")
            nc.vector.tensor_copy(rrow_b[:], rrow[:])
            kvs = []
            for i, Wk in ((0, Wk1b), (1, Wk2b)):
                psK = mm_tile(128, 128)
                nc.tensor.matmul(psK, Wk, X1b[:], start=True, stop=False)
                nc.tensor.matmul(psK, rowb[:1, i * 128:(i + 1) * 128],
                                 rrow_b[:], start=False, stop=True)
                kv = wtile([128, 128], bf16, f"k{i+1}v_b")
                nc.vector.tensor_copy(kv[:], psK)
                kvs.append(kv)
            psW = mm_tile(128, 128)
            nc.tensor.matmul(psW, Wq1Tb, kvs[0][:], start=True, stop=False)
            nc.tensor.matmul(psW, Wq2Tsb, kvs[1][:], start=False, stop=True)
            Wattb = wtile([128, D], bf16, "Wattb")
            nc.vector.tensor_copy(Wattb[:], psW)
            psB = mm_tile(128, 1)
            nc.tensor.matmul(psB, kvs[0][:], colb[:, 0:1], start=True,
                             stop=False)
            nc.tensor.matmul(psB, kvs[1][:], colb[:, 1:2], start=False,
                             stop=True)
            batt_c = wtile([128, 1], f32, "batt_c")
            nc.vector.tensor_copy(batt_c[:], psB)
            # sT = Watt^T @ hnT + batt  (transposed layout)
            for bo, bw in BLK:
                pss = p2_tile(bw)
                nc.tensor.matmul(pss, Wattb[:], hnTb[:, bo:bo + bw])
                nc.vector.tensor_scalar(sT[:, bo:bo + bw], pss,
                                        scalar1=batt_c[:], scalar2=None,
                                        op0=ALU.add)
                nc.vector.tensor_copy(sTb[:, bo:bo + bw], sT[:, bo:bo + bw])
            lnT(sT, aTb, "s", xb=sTb)
            # haT += Wo'^T @ aT + bo   (commutative accumulation)
            for bo, bw in BLK:
                psa = p2_tile(bw)
                nc.tensor.matmul(psa, Wob, aTb[:, bo:bo + bw])
                atmp = rowtmp.tile([128, 512], f32, tag="btmp", bufs=2,
                                   name=f"atmp_{nc.next_id()}")[:, :bw]
                nc.vector.tensor_scalar(atmp, psa, scalar1=bo_c,
                                        scalar2=None, op0=ALU.add)
                nc.vector.tensor_add(haT[:, bo:bo + bw],
                                     haT[:, bo:bo + bw], atmp)

        # ---- pipeline ----
        emit_pass1(0, mid=emit_lngram)
        for _c in range(1, NCH):
            emit_pass1(_c)
        for _c in range(NCH):
            emit_uT_loads(_c)
        # haT starts as a copy of hT; att and pass2 then += into it
        nc.vector.tensor_copy(haT[:], hT[:])
        lnT(hT, hnTb, "hn", xb=hTb)   # feeds sT; runs in the AR0 wait gap
        emit_att()
        for _c in range(NCH):
            emit_pass2(_c)

        # ======= final epilogue: fused per-block LN+FFN, early out DMA =====
        outT = sT  # sT is dead after lnT(sT); reuse its buffer
        fxb = wpool.tile([128, NLOC], bf16, tag="hTb", name="fxb")
        fx2 = wpool.tile([128, NLOC], bf16, tag="ln_x2b", name="fx2")
        mrow = rowtmp.tile([1, NLOC], f32, tag="ep_m", bufs=1, name="ep_m")
        vrow = rowtmp.tile([1, NLOC], f32, tag="ep_v", bufs=1, name="ep_v")
        for bo, bw in BLK:
            nc.scalar.activation(fxb[:, bo:bo + bw], haT[:, bo:bo + bw],
                                 AF.Copy)
            nc.vector.tensor_mul(fx2[:, bo:bo + bw], haT[:, bo:bo + bw],
                                 haT[:, bo:bo + bw])
            psm = mm_tile(1, bw)
            nc.tensor.matmul(psm, oinv_col_b[:], fxb[:, bo:bo + bw])
            nc.vector.tensor_copy(mrow[:, bo:bo + bw], psm)
            psq_ = mm_tile(1, bw)
            nc.tensor.matmul(psq_, oinv_col_b[:], fx2[:, bo:bo + bw])
            nc.vector.tensor_mul(vrow[:, bo:bo + bw], mrow[:, bo:bo + bw],
                                 mrow[:, bo:bo + bw])
            nc.vector.tensor_sub(vrow[:, bo:bo + bw], psq_,
                                 vrow[:, bo:bo + bw])
        nc.scalar.activation(vrow[:], vrow[:], AF.Sqrt, bias=eps_col[:1])
        nc.vector.reciprocal(vrow[:], vrow[:])
        m_bf = rowtmp.tile([1, NLOC], bf16, tag="ep_mb", bufs=1, name="ep_mb")
        nc.vector.tensor_copy(m_bf[:], mrow[:])
        rs_bf = rowtmp.tile([1, NLOC], bf16, tag="ep_rb", bufs=1,
                            name="ep_rb")
        nc.vector.tensor_copy(rs_bf[:], vrow[:])
        stats = [(m_bf[:, bo:bo + bw], rs_bf[:, bo:bo + bw])
                 for bo, bw in BLK]
        for (bo, bw), (m_b, rs_b) in zip(BLK, stats):
            psM = p2_tile(bw)
            nc.tensor.matmul(psM, ones_row_b[:], m_b)
            psR = p2_tile(bw)
            nc.tensor.matmul(psR, ones_row_b[:], rs_b)
            dtmp = rowtmp.tile([128, 512], f32, tag="btmp", bufs=2,
                               name=f"lnd_{nc.next_id()}")[:, :bw]
            nc.vector.tensor_sub(dtmp, haT[:, bo:bo + bw], psM)
            fb_ = rtile([128, 512], bf16, "fb")[:, :bw]
            nc.vector.tensor_mul(fb_, dtmp, psR)
            psg_ = p2_tile(bw)
            nc.tensor.matmul(psg_, W1pb, fb_)
            gb_ = rtile([128, 512], bf16, "gb")[:, :bw]
            nc.scalar.activation(gb_, psg_, AF.Gelu, bias=b1p_c[:])
            pso_ = mm_tile(128, bw)
            nc.tensor.matmul(pso_, f2wb, gb_)
            otmp = rowtmp.tile([128, 512], f32, tag="btmp", bufs=2,
                               name=f"otmp_{nc.next_id()}")[:, :bw]
            nc.vector.tensor_scalar(otmp, pso_, scalar1=f2b_c,
                                    scalar2=None, op0=ALU.add)
            nc.vector.tensor_add(outT[:, bo:bo + bw],
                                 haT[:, bo:bo + bw], otmp)
            nc.sync.dma_start(out=t_out[:, bo:bo + bw],
                              in_=outT[:, bo:bo + bw])

    nc.compile()
    return nc


# ==================== host-side entry point ====================

_CACHED = {}


def _get_nc(N=N_FULL, NF=NF_FULL, CORES=CORES_FULL):
    key = (N, NF, CORES)
    if key not in _CACHED:
        _CACHED[key] = build_kernel(N, NF, CORES)
    return _CACHED[key]


def make_in_maps(inputs, N, CORES):
    import ml_dtypes

    NLOC = N // CORES
    NSUB = (N + 127) // 128
    bf = ml_dtypes.bfloat16
    f = {k: np.asarray(v, np.float64) for k, v in inputs.items()}
    LI = LAMBDA_INIT

    lam1 = np.exp(np.sum(f["lq1"] * f["lk1"]))
    lam2 = np.exp(np.sum(f["lq2"] * f["lk2"]))
    lam = lam1 - lam2 + LI
    mg, mb = f["mha_ln_g"], f["mha_ln_b"]
    Wk1 = f["k1_w"] * mg[:, None]; bk1 = mb @ f["k1_w"] + f["k1_b"]
    Wk2 = f["k2_w"] * mg[:, None]; bk2 = mb @ f["k2_w"] + f["k2_b"]
    Wv = f["v_w"] * mg[:, None]; bv = mb @ f["v_w"] + f["v_b"]
    Wq1 = f["q1_w"] * mg[:, None]; bq1 = mb @ f["q1_w"] + f["q1_b"]
    Wq2 = f["q2_w"] * mg[:, None]; bq2 = mb @ f["q2_w"] + f["q2_b"]
    Wob = f["attn_ln_g"][:, None] * f["out_w"] * (1 - LI)
    bo = (1 - LI) * (f["attn_ln_b"] @ f["out_w"]) + f["out_b"]
    W1p = f["ffn_ln_g"][:, None] * f["ffn1_w"]
    b1p = f["ffn_ln_b"] @ f["ffn1_w"] + f["ffn1_b"]

    kk = np.arange(1, 11)
    ang = f["e"][:, None] * kk / np.pi
    ne = (np.cos(ang) @ f["kan_a"] + np.sin(ang) @ f["kan_b"]
          + f["kan_bias"][0]) * f["alpha_w"][0, 0]
    ne_pad = np.zeros(NSUB * 128)
    ne_pad[:N] = ne
    ne_pm = np.ascontiguousarray(
        ne_pad.reshape(NSUB, 128).T.astype(np.float32))

    wb = np.concatenate(
        [f["fe_w1"].reshape(4, 128, 128)[i] for i in range(4)]
        + [f["fe_w2"], Wk1, Wk2, Wv, Wq1.T, -lam * Wq2.T, Wob, W1p,
           f["ffn2_w"]], axis=1)
    wb = np.ascontiguousarray(wb.astype(bf))
    colw = np.stack([f["fe_b1"], f["fe_b2"], bo, b1p, f["ffn2_b"],
                     np.zeros(128), np.zeros(128), np.zeros(128)], axis=1)
    colw = np.ascontiguousarray(colw.astype(np.float32))
    colb = np.ascontiguousarray(
        np.stack([bq1, -lam * bq2], axis=1).astype(bf))
    rowb = np.ascontiguousarray(
        np.concatenate([bk1, bk2, bv, np.zeros(128)])[None, :].astype(bf))
    roww = np.ascontiguousarray((N * bv)[None, :].astype(np.float32))

    x = np.asarray(inputs["x"], np.float32)
    u = np.asarray(inputs["u"], np.float32)
    in_maps = []
    for c in range(CORES):
        sh = u[c * NLOC:(c + 1) * NLOC]
        m = {
            "u": np.ascontiguousarray(sh.astype(bf)),
            "uT": np.ascontiguousarray(sh.T.astype(bf)),
            "xT": np.ascontiguousarray(
                x[c * NLOC:(c + 1) * NLOC].T.astype(bf)),
            "wb": wb, "colw": colw, "colb": colb, "rowb": rowb,
            "roww": roww, "ne": ne_pm,
        }
        in_maps.append(m)
    return in_maps


def assemble_out(res, CORES=CORES_FULL):
    # per-core outputs are [D, NLOC] (transposed); transpose + concat rows
    return np.concatenate(
        [np.asarray(res.results[c]["out"]).T for c in range(CORES)],
        axis=0).astype(np.float32)


def kernel(**inputs):
    from concourse import bass_utils

    nc = _get_nc()
    in_maps = make_in_maps(inputs, N_FULL, CORES_FULL)
    res = bass_utils.run_bass_kernel_spmd(nc, in_maps,
                                          core_ids=list(range(CORES_FULL)))
    return assemble_out(res)


if __name__ == "__main__":
    build_kernel()
    print("build ok")


# revision 47
# speedup vs baseline: 1.1498x; 1.0396x over previous
"""Trainium2 Bass kernel for nn_NoFoDifformer_FourierKAN (8-core SPMD), v4.

Sharding: u and nodes row-wise across 8 cores (1250 rows each). The z = u^T h
partial sums are all-reduced per column-chunk (bf16); the [d,d] Gram matrix of
the normalized activations rides inside the first chunk's all-reduce. Small
weights are pre-folded and packed on the HOST (LayerNorm affines folded into
projections, lambda and the FourierKAN filter evaluated in numpy) so the
device preamble is a handful of panel DMAs. Per-core outputs are [d, n_loc]
(transposed) and transposed+concatenated on the host.

Scheduling notes (engine FIFOs are in-order; a blocked DMA trigger blocks
everything behind it on that queue, so queue assignment is load-bearing):
- sync queue: xT, all u quarter-tiles, then all uT tiles, zc readbacks, out
  blocks. uT triggers drain only after the u triggers, so uT prefetch can
  never race the pass1-feeding u stream.
- scalar queue: activations + pass1 PSUM->staging casts (no uT triggers ->
  the LN Sqrt chain can never deadlock against uT pool recycling).
- gpsimd queue: weight panels, p1 staging writes, AR triggers. AR triggers
  serialize on the previous AR's completion, keeping the chain dense.
- ASCENDING chunk plan [2048, 4096, 3856]: the small chunk 0 puts the first
  AllReduce right at the end of the startup barrier (the AR chain is the
  critical path: ~20 GB/s alg ring bandwidth); the big middle ARs hide
  pass2/uT streaming; lnT+gram are emitted inside pass1(0) so the Gram
  rides chunk 0's AR with minimal doorbell delay.
- attention epilogue (watt/sT/haT) is emitted after the pass1 stages and
  accumulates commutatively into haT (initialized as a copy of hT).
"""

import numpy as np

N_FULL = 10000
NF_FULL = 512
D = 128
CORES_FULL = 8
LAMBDA_INIT = 0.2
GEXT = 256  # extra AR columns on chunk 0 for (G | s)
CH_PLAN = [2048, 4992, 2960]


def _splits(total, step):
    return [(o, min(step, total - o)) for o in range(0, total, step)]


def build_kernel(N=N_FULL, NF=NF_FULL, CORES=CORES_FULL):
    import concourse.bacc as bacc
    import concourse.tile as tile
    from concourse import mybir
    from concourse.masks import make_identity
    from contextlib import ExitStack

    dt = mybir.dt
    f32 = dt.float32
    bf16 = dt.bfloat16
    AF = mybir.ActivationFunctionType
    ALU = mybir.AluOpType

    NLOC = N // CORES                   # 1250
    ROWS = _splits(NLOC, 128)           # 9x128 + 98
    NT = len(ROWS)
    KX = NF // 128
    assert sum(CH_PLAN) == N
    assert all(w % 128 == 0 for w in CH_PLAN[:-1])
    CHUNKS = []
    off = 0
    for w in CH_PLAN:
        CHUNKS.append((off, w))
        off += w
    NCH = len(CHUNKS)
    CHMAX = max(CH_PLAN)
    NSUB = (N + 127) // 128             # 79
    BLK = _splits(NLOC, 512)            # [d, NLOC] op blocks
    rg = [list(range(CORES))]
    shared_space = "Shared" if CORES > 4 else "Local"

    nc = bacc.Bacc("TRN2", target_bir_lowering=False, debug=False,
                   num_devices=CORES)

    # ---------------- DRAM I/O ----------------
    t_u = nc.dram_tensor("u", [NLOC, N], bf16, kind="ExternalInput")
    t_uT = nc.dram_tensor("uT", [N, NLOC], bf16, kind="ExternalInput")
    t_xT = nc.dram_tensor("xT", [NF, NLOC], bf16, kind="ExternalInput")
    t_wb = nc.dram_tensor("wb", [128, 13 * 128], bf16, kind="ExternalInput")
    t_colw = nc.dram_tensor("colw", [128, 8], f32, kind="ExternalInput")
    t_colb = nc.dram_tensor("colb", [128, 2], bf16, kind="ExternalInput")
    t_rowb = nc.dram_tensor("rowb", [1, 4 * 128], bf16, kind="ExternalInput")
    t_roww = nc.dram_tensor("roww", [1, 128], f32, kind="ExternalInput")
    t_ne = nc.dram_tensor("ne", [128, NSUB], f32, kind="ExternalInput")
    t_out = nc.dram_tensor("out", [D, NLOC], f32, kind="ExternalOutput")

    with tile.TileContext(nc) as tc, ExitStack() as ctx:
        wpool = ctx.enter_context(tc.tile_pool(name="wpool", bufs=1))
        rowtmp = ctx.enter_context(tc.tile_pool(name="rowtmp", bufs=3))
        ustream = ctx.enter_context(tc.tile_pool(name="ustream", bufs=15))
        uTp = ctx.enter_context(tc.tile_pool(name="uTp", bufs=18))
        zcp = ctx.enter_context(tc.tile_pool(name="zcp", bufs=1))
        z16p = ctx.enter_context(tc.tile_pool(name="z16p", bufs=1))
        p1sbp = ctx.enter_context(tc.tile_pool(name="p1sbp", bufs=1))
        dram = ctx.enter_context(tc.tile_pool(name="dram", bufs=1, space="DRAM"))
        ps_p1 = ctx.enter_context(tc.tile_pool(name="ps_p1", bufs=2, space="PSUM"))
        ps_p2 = ctx.enter_context(tc.tile_pool(name="ps_p2", bufs=3, space="PSUM"))
        ps_mm = ctx.enter_context(tc.tile_pool(name="ps_mm", bufs=1, space="PSUM"))
        ps_t = ctx.enter_context(tc.tile_pool(name="ps_t", bufs=2, space="PSUM"))

        def p1_tile(w):
            return ps_p1.tile([128, 512], f32, tag="p1",
                              name=f"p1_{nc.next_id()}")[:, :w]

        def p2_tile(w):
            return ps_p2.tile([128, 512], f32, tag="p2",
                              name=f"p2_{nc.next_id()}")[:, :w]

        def mm_tile(p, w):
            return ps_mm.tile([128, 512], f32, tag="mmp",
                              name=f"mm_{nc.next_id()}")[:p, :w]

        def tb_tile(p, w):
            return ps_t.tile([128, 128], bf16, tag="pstb",
                             name=f"pstb_{nc.next_id()}")[:p, :w]

        def wtile(shape, dtype, name):
            return wpool.tile(shape, dtype, tag=name, name=name)

        def rtile(shape, dtype, tag):
            return rowtmp.tile(shape, dtype, tag=tag,
                               name=f"{tag}_{nc.next_id()}")

        def T(out_psum, in_sbuf, identity):
            nc.tensor.matmul(out_psum, in_sbuf, identity, is_transpose=True)

        # ================= constants & weights =================
        identb = wtile([128, 128], bf16, "identb")
        make_identity(nc, identb[:])
        ones_row_b = wtile([1, 128], bf16, "ones_row_b")
        nc.vector.memset(ones_row_b[:], 1.0)
        ones_col_b = wtile([128, 1], bf16, "ones_col_b")
        nc.vector.memset(ones_col_b[:], 1.0)
        oinv_col_b = wtile([128, 1], bf16, "oinv_col_b")
        nc.vector.memset(oinv_col_b[:], 1.0 / 128.0)
        eps_col = wtile([128, 1], f32, "eps_col")
        nc.vector.memset(eps_col[:], 1e-5)

        wb = wtile([128, 13 * 128], bf16, "wb")
        nc.gpsimd.dma_start(out=wb[:], in_=t_wb[:])
        colw = wtile([128, 8], f32, "colw")
        nc.gpsimd.dma_start(out=colw[:], in_=t_colw[:])
        colb = wtile([128, 2], bf16, "colb")
        nc.gpsimd.dma_start(out=colb[:], in_=t_colb[:])
        rowb = wtile([1, 4 * 128], bf16, "rowb")
        nc.gpsimd.dma_start(out=rowb[:], in_=t_rowb[:])
        roww = wtile([1, 128], f32, "roww")
        nc.gpsimd.dma_start(out=roww[:], in_=t_roww[:])
        ne = wtile([128, NSUB], f32, "ne")
        nc.gpsimd.dma_start(out=ne[:], in_=t_ne[:])

        def P(i):  # weight panel i of wb
            return wb[:, i * 128:(i + 1) * 128]
        few2b = P(4)
        Wk1b, Wk2b, Wvb = P(5), P(6), P(7)
        Wq1Tb, Wq2Tsb = P(8), P(9)
        Wob, W1pb, f2wb = P(10), P(11), P(12)
        feb1_c = colw[:, 0:1]
        feb2_c = colw[:, 1:2]
        bo_c = colw[:, 2:3]
        b1p_c = colw[:, 3:4]
        f2b_c = colw[:, 4:5]

        # xT into SBUF (4 partition k-tiles)
        xT4 = wtile([128, KX, NLOC], bf16, "xT4")
        for kt in range(KX):
            nc.sync.dma_start(out=xT4[:, kt, :],
                              in_=t_xT[kt * 128:(kt + 1) * 128, :])

        # ---------- u streaming loads (quarter tiles, sync queue) ----------
        u_tiles = {}

        def emit_u_loads(c):
            co, cw = CHUNKS[c]
            tiles = {}
            for q, (qo, qw) in enumerate(_splits(cw, 1024)):
                for r, (ro, rw) in enumerate(ROWS):
                    ut = ustream.tile([128, 1024], bf16, tag="u",
                                      name=f"u{c}_{q}_{r}")[:rw, :qw]
                    nc.sync.dma_start(
                        out=ut, in_=t_u[ro:ro + rw, co + qo:co + qo + qw])
                    tiles[(q, r)] = ut
            u_tiles[c] = tiles

        uT_tiles = {}

        def emit_uT_loads(c, eng=None):
            co, cw = CHUNKS[c]
            tl = []
            for t, (so, sw) in enumerate(_splits(cw, 128)):
                uTt = uTp.tile([128, NLOC], bf16, tag="uT",
                               name=f"uTl{c}_{t}")[:sw]
                (eng or nc.sync).dma_start(
                    out=uTt, in_=t_uT[co + so:co + so + sw, :])
                tl.append(uTt)
            uT_tiles[c] = tl

        for _c in range(NCH):
            emit_u_loads(_c)

        # ================= phase A: feature encoder (transposed) ==========
        hT = wtile([128, NLOC], f32, "hT")
        hTb = wtile([128, NLOC], bf16, "hTb")
        h16 = wtile([128, NT, D], bf16, "h16")
        for go, gw in BLK:
            psh1 = p2_tile(gw)
            for kt in range(KX):
                nc.tensor.matmul(psh1, P(kt), xT4[:, kt, go:go + gw],
                                 start=(kt == 0), stop=(kt == KX - 1))
            h1t = rtile([128, 512], bf16, "h1t")[:, :gw]
            nc.scalar.activation(h1t, psh1, AF.Relu, bias=feb1_c[:])
            pshT = p2_tile(gw)
            nc.tensor.matmul(pshT, few2b, h1t)
            nc.vector.tensor_scalar(hT[:, go:go + gw], pshT,
                                    scalar1=feb2_c, scalar2=None, op0=ALU.add)
            nc.scalar.activation(hTb[:, go:go + gw], hT[:, go:go + gw],
                                 AF.Copy)
        for r, (ro, rw) in enumerate(ROWS):
            pst = tb_tile(rw, 128)
            T(pst, hTb[:, ro:ro + rw], identb[:])
            nc.vector.tensor_copy(h16[:rw, r, :], pst)

        # ---------- transposed-layout LayerNorm helper ----------
        def lnT(x_sb, out_bf, pfx, xb=None):
            if xb is None:
                xb = wpool.tile([128, NLOC], bf16, tag="ln_xb",
                                name=f"{pfx}_xb")
                nc.scalar.activation(xb[:], x_sb[:], AF.Copy)
            x2b = wpool.tile([128, NLOC], bf16, tag="ln_x2b",
                             name=f"{pfx}_x2b")
            nc.vector.tensor_mul(x2b[:], x_sb[:], x_sb[:])

            def frow(tag, dt_):
                return rowtmp.tile([1, NLOC], dt_, tag=tag, bufs=2,
                                   name=f"{tag}_{nc.next_id()}")
            mrow = frow("ln_m", f32)
            rsrow = frow("ln_r", f32)
            for bo, bw in BLK:
                psm = mm_tile(1, bw)
                nc.tensor.matmul(psm, oinv_col_b[:], xb[:, bo:bo + bw])
                nc.vector.tensor_copy(mrow[:, bo:bo + bw], psm)
                psq_ = mm_tile(1, bw)
                nc.tensor.matmul(psq_, oinv_col_b[:], x2b[:, bo:bo + bw])
                nc.vector.tensor_mul(rsrow[:, bo:bo + bw],
                                     mrow[:, bo:bo + bw], mrow[:, bo:bo + bw])
                nc.vector.tensor_sub(rsrow[:, bo:bo + bw], psq_,
                                     rsrow[:, bo:bo + bw])         # var
            nc.scalar.activation(rsrow[:], rsrow[:], AF.Sqrt,
                                 bias=eps_col[:1])
            nc.vector.reciprocal(rsrow[:], rsrow[:])               # 1/sqrt
            m_b = frow("ln_mb", bf16)
            nc.vector.tensor_copy(m_b[:], mrow[:])
            rs_b = frow("ln_rb", bf16)
            nc.vector.tensor_copy(rs_b[:], rsrow[:])
            for bo, bw in BLK:
                psM = p2_tile(bw)
                nc.tensor.matmul(psM, ones_row_b[:], m_b[:, bo:bo + bw])
                psR = p2_tile(bw)
                nc.tensor.matmul(psR, ones_row_b[:], rs_b[:, bo:bo + bw])
                dtmp = rowtmp.tile([128, 512], f32, tag="btmp", bufs=2,
                                   name=f"lnd_{nc.next_id()}")[:, :bw]
                nc.vector.tensor_sub(dtmp, x_sb[:, bo:bo + bw], psM)
                nc.vector.tensor_mul(out_bf[:, bo:bo + bw], dtmp, psR)

        hnTb = wtile([128, NLOC], bf16, "hnTb")
        gram_sb = wtile([128, GEXT], bf16, "gram_sb")

        def emit_lngram():
            # Gram straight from h16 row tiles: per-row bn_stats LN (tiny
            # [rw,1] reciprocals, no transposes) so G = hn^T hn, s = hn^T 1
            # are ready ~30us earlier than via the full-width lnT. The
            # transposed hnTb for the attention path is produced later by
            # lnT(hT) off the AR0 critical path.
            psGS = ps_mm.tile([128, 512], f32, tag="mmp", name="psGS")
            for r, (ro, rw) in enumerate(ROWS):
                stats = rtile([128, 6], f32, "gst")
                nc.vector.bn_stats(stats[:rw], h16[:rw, r, :])
                mv = rtile([128, 2], f32, "gmv")
                nc.vector.bn_aggr(mv[:rw], stats[:rw])
                rs = rtile([128, 1], f32, "grs")
                nc.scalar.activation(rs[:rw], mv[:rw, 1:2], AF.Sqrt,
                                     bias=eps_col[:rw])
                nc.vector.reciprocal(rs[:rw], rs[:rw])
                hn_r = rtile([128, 128], bf16, "hn_r")[:rw]
                nc.vector.tensor_scalar(hn_r, h16[:rw, r, :],
                                        scalar1=mv[:rw, 0:1],
                                        op0=ALU.subtract,
                                        scalar2=rs[:rw], op1=ALU.mult)
                nc.tensor.matmul(psGS[:, 0:128], hn_r, hn_r,
                                 start=(r == 0), stop=(r == NT - 1))
                nc.tensor.matmul(psGS[:1, 128:256], ones_col_b[:rw], hn_r,
                                 start=(r == 0), stop=(r == NT - 1))
            nc.vector.tensor_copy(gram_sb[:, 0:128], psGS[:, 0:128])
            nc.vector.tensor_copy(gram_sb[:1, 128:256], psGS[:1, 128:256])

        # ---------- DRAM staging ----------
        p1_in, p1_out = [], []
        for c, (co, cw) in enumerate(CHUNKS):
            w = cw + (GEXT if c == 0 else 0)
            p1_in.append(dram.tile([128, w], bf16, tag=f"p1in{c}",
                                   name=f"p1in{c}"))
            p1_out.append(dram.tile([128, w], bf16, tag=f"p1out{c}",
                                    name=f"p1out{c}", addr_space=shared_space))

        haT = wtile([128, NLOC], f32, "haT")
        sT = wtile([128, NLOC], f32, "sT")
        sTb = wpool.tile([128, NLOC], bf16, tag="hTb", name="sTb")
        aTb = wpool.tile([128, NLOC], bf16, tag="hnTb", name="aTb")

        def emit_pass1(c, mid=None):
            co, cw = CHUNKS[c]
            ut = u_tiles.pop(c)
            w = cw + (GEXT if c == 0 else 0)
            p1sb = p1sbp.tile([128, CHMAX + GEXT], bf16, tag="p1sb",
                              name=f"p1sb{c}")[:, :w]
            for q, (qo, qw) in enumerate(_splits(cw, 1024)):
                blocks = _splits(qw, 512)
                ps1 = [p1_tile(bw) for _, bw in blocks]
                for r, (ro, rw) in enumerate(ROWS):
                    for bi, (bo, bw) in enumerate(blocks):
                        nc.tensor.matmul(ps1[bi], h16[:rw, r, :],
                                         ut[(q, r)][:rw, bo:bo + bw],
                                         start=(r == 0), stop=(r == NT - 1))
                if q == 0 and mid is not None:
                    mid()
                for bi, (bo, bw) in enumerate(blocks):
                    nc.scalar.activation(p1sb[:, qo + bo:qo + bo + bw],
                                         ps1[bi], AF.Copy)
            if c == 0:
                nc.scalar.activation(p1sb[:, cw:cw + GEXT], gram_sb[:],
                                     AF.Copy)
            nc.gpsimd.dma_start(out=p1_in[c][:], in_=p1sb)
            nc.gpsimd.collective_compute(
                "AllReduce", ALU.add, replica_groups=rg,
                ins=[p1_in[c].opt()], outs=[p1_out[c].opt()])

        def emit_pass2(c):
            co, cw = CHUNKS[c]
            subs = _splits(cw, 128)
            zc = zcp.tile([128, CHMAX], bf16, tag="zc",
                          name=f"zc_{c}")[:, :cw]
            nc.scalar.dma_start(out=zc, in_=p1_out[c][:, :cw])
            z16 = z16p.tile([128, (CHMAX + 127) // 128, D], bf16, tag="z16",
                            name=f"z16_{c}")
            for t, (so, sw) in enumerate(subs):
                psz = tb_tile(sw, 128)
                T(psz, zc[:, so:so + sw], identb[:])
                gidx = (co + so) // 128
                nc.vector.tensor_scalar(z16[:sw, t, :], psz,
                                        scalar1=ne[:sw, gidx:gidx + 1],
                                        scalar2=None, op0=ALU.mult)
            uTc = uT_tiles.pop(c)
            ps2 = [p2_tile(iw) for _, iw in BLK]
            for t, (so, sw) in enumerate(subs):
                for ib, (io, iw) in enumerate(BLK):
                    nc.tensor.matmul(ps2[ib], z16[:sw, t, :],
                                     uTc[t][:sw, io:io + iw],
                                     start=(t == 0), stop=(t == len(subs) - 1))
            for ib, (io, iw) in enumerate(BLK):
                nc.vector.tensor_add(haT[:, io:io + iw],
                                     haT[:, io:io + iw], ps2[ib])

        def emit_att():
            # post-AR0 attention path: gram -> k1v/k2v -> Watt -> sT -> aT -> haT
            co0, cw0 = CHUNKS[0]
            gkv = wtile([128, GEXT], bf16, "gkv")
            nc.scalar.dma_start(out=gkv[:], in_=p1_out[0][:, cw0:cw0 + GEXT])
            G_b = gkv[:, 0:128]
            s_row = gkv[:1, 128:256]
            psc = tb_tile(128, 1)
            T(psc, s_row, identb[:1, :1])
            s_col = rtile([128, 1], bf16, "s_col")
            nc.vector.tensor_copy(s_col[:], psc)
            # X1 = G Wv + s (.) bv   (shared by k1v and k2v)
            psX = mm_tile(128, 128)
            nc.tensor.matmul(psX, G_b, Wvb, start=True, stop=False)
            nc.tensor.matmul(psX, s_row, rowb[:1, 256:384], start=False,
                             stop=True)
            X1b = wtile([128, 128], bf16, "X1b")
            nc.vector.tensor_copy(X1b[:], psX)
            # rrow = s^T Wv + N bv
            psr = mm_tile(1, 128)
            nc.tensor.matmul(psr, s_col[:], Wvb)
            rrow = rtile([1, 128], f32, "rrow")
            nc.vector.tensor_add(rrow[:], psr, roww[:1])
            rrow_b = rtile([1, 128], bf16, "rrow_b")
            nc.vector.tensor_copy(rrow_b[:], rrow[:])
            kvs = []
            for i, Wk in ((0, Wk1b), (1, Wk2b)):
                psK = mm_tile(128, 128)
                nc.tensor.matmul(psK, Wk, X1b[:], start=True, stop=False)
                nc.tensor.matmul(psK, rowb[:1, i * 128:(i + 1) * 128],
                                 rrow_b[:], start=False, stop=True)
                kv = wtile([128, 128], bf16, f"k{i+1}v_b")
                nc.vector.tensor_copy(kv[:], psK)
                kvs.append(kv)
            psW = mm_tile(128, 128)
            nc.tensor.matmul(psW, Wq1Tb, kvs[0][:], start=True, stop=False)
            nc.tensor.matmul(psW, Wq2Tsb, kvs[1][:], start=False, stop=True)
            Wattb = wtile([128, D], bf16, "Wattb")
            nc.vector.tensor_copy(Wattb[:], psW)
            psB = mm_tile(128, 1)
            nc.tensor.matmul(psB, kvs[0][:], colb[:, 0:1], start=True,
                             stop=False)
            nc.tensor.matmul(psB, kvs[1][:], colb[:, 1:2], start=False,
                             stop=True)
            batt_c = wtile([128, 1], f32, "batt_c")
            nc.vector.tensor_copy(batt_c[:], psB)
            # sT = Watt^T @ hnT + batt  (transposed layout)
            for bo, bw in BLK:
                pss = p2_tile(bw)
                nc.tensor.matmul(pss, Wattb[:], hnTb[:, bo:bo + bw])
                nc.vector.tensor_scalar(sT[:, bo:bo + bw], pss,
                                        scalar1=batt_c[:], scalar2=None,
                                        op0=ALU.add)
                nc.vector.tensor_copy(sTb[:, bo:bo + bw], sT[:, bo:bo + bw])
            lnT(sT, aTb, "s", xb=sTb)
            # haT += Wo'^T @ aT + bo   (commutative accumulation)
            for bo, bw in BLK:
                psa = p2_tile(bw)
                nc.tensor.matmul(psa, Wob, aTb[:, bo:bo + bw])
                atmp = rowtmp.tile([128, 512], f32, tag="btmp", bufs=2,
                                   name=f"atmp_{nc.next_id()}")[:, :bw]
                nc.vector.tensor_scalar(atmp, psa, scalar1=bo_c,
                                        scalar2=None, op0=ALU.add)
                nc.vector.tensor_add(haT[:, bo:bo + bw],
                                     haT[:, bo:bo + bw], atmp)

        # ---- pipeline ----
        emit_pass1(0, mid=emit_lngram)
        for _c in range(1, NCH):
            emit_pass1(_c)
        for _c in range(NCH):
            emit_uT_loads(_c)
        # haT starts as a copy of hT; att and pass2 then += into it
        nc.vector.tensor_copy(haT[:], hT[:])
        lnT(hT, hnTb, "hn", xb=hTb)   # feeds sT; runs in the AR0 wait gap
        emit_att()
        for _c in range(NCH):
            emit_pass2(_c)

        # ======= final epilogue: fused per-block LN+FFN, early out DMA =====
        outT = sT  # sT is dead after lnT(sT); reuse its buffer
        fxb = wpool.tile([128, NLOC], bf16, tag="hTb", name="fxb")
        fx2 = wpool.tile([128, NLOC], bf16, tag="ln_x2b", name="fx2")
        mrow = rowtmp.tile([1, NLOC], f32, tag="ep_m", bufs=1, name="ep_m")
        vrow = rowtmp.tile([1, NLOC], f32, tag="ep_v", bufs=1, name="ep_v")
        for bo, bw in BLK:
            nc.scalar.activation(fxb[:, bo:bo + bw], haT[:, bo:bo + bw],
                                 AF.Copy)
            nc.vector.tensor_mul(fx2[:, bo:bo + bw], haT[:, bo:bo + bw],
                                 haT[:, bo:bo + bw])
            psm = mm_tile(1, bw)
            nc.tensor.matmul(psm, oinv_col_b[:], fxb[:, bo:bo + bw])
            nc.vector.tensor_copy(mrow[:, bo:bo + bw], psm)
            psq_ = mm_tile(1, bw)
            nc.tensor.matmul(psq_, oinv_col_b[:], fx2[:, bo:bo + bw])
            nc.vector.tensor_mul(vrow[:, bo:bo + bw], mrow[:, bo:bo + bw],
                                 mrow[:, bo:bo + bw])
            nc.vector.tensor_sub(vrow[:, bo:bo + bw], psq_,
                                 vrow[:, bo:bo + bw])
        nc.scalar.activation(vrow[:], vrow[:], AF.Sqrt, bias=eps_col[:1])
        nc.vector.reciprocal(vrow[:], vrow[:])
        m_bf = rowtmp.tile([1, NLOC], bf16, tag="ep_mb", bufs=1, name="ep_mb")
        nc.vector.tensor_copy(m_bf[:], mrow[:])
        rs_bf = rowtmp.tile([1, NLOC], bf16, tag="ep_rb", bufs=1,
                            name="ep_rb")
        nc.vector.tensor_copy(rs_bf[:], vrow[:])
        stats = [(m_bf[:, bo:bo + bw], rs_bf[:, bo:bo + bw])
                 for bo, bw in BLK]
        for (bo, bw), (m_b, rs_b) in zip(BLK, stats):
            psM = p2_tile(bw)
            nc.tensor.matmul(psM, ones_row_b[:], m_b)
            psR = p2_tile(bw)
            nc.tensor.matmul(psR, ones_row_b[:], rs_b)
            dtmp = rowtmp.tile([128, 512], f32, tag="btmp", bufs=2,
                               name=f"lnd_{nc.next_id()}")[:, :bw]
            nc.vector.tensor_sub(dtmp, haT[:, bo:bo + bw], psM)
            fb_ = rtile([128, 512], bf16, "fb")[:, :bw]
            nc.vector.tensor_mul(fb_, dtmp, psR)
            psg_ = p2_tile(bw)
            nc.tensor.matmul(psg_, W1pb, fb_)
            gb_ = rtile([128, 512], bf16, "gb")[:, :bw]
            nc.scalar.activation(gb_, psg_, AF.Gelu, bias=b1p_c[:])
            pso_ = mm_tile(128, bw)
            nc.tensor.matmul(pso_, f2wb, gb_)
            otmp = rowtmp.tile([128, 512], f32, tag="btmp", bufs=2,
                               name=f"otmp_{nc.next_id()}")[:, :bw]
            nc.vector.tensor_scalar(otmp, pso_, scalar1=f2b_c,
                                    scalar2=None, op0=ALU.add)
            nc.vector.tensor_add(outT[:, bo:bo + bw],
                                 haT[:, bo:bo + bw], otmp)
            nc.sync.dma_start(out=t_out[:, bo:bo + bw],
                              in_=outT[:, bo:bo + bw])

    nc.compile()
    return nc


# ==================== host-side entry point ====================

_CACHED = {}


def _get_nc(N=N_FULL, NF=NF_FULL, CORES=CORES_FULL):
    key = (N, NF, CORES)
    if key not in _CACHED:
        _CACHED[key] = build_kernel(N, NF, CORES)
    return _CACHED[key]


def make_in_maps(inputs, N, CORES):
    import ml_dtypes

    NLOC = N // CORES
    NSUB = (N + 127) // 128
    bf = ml_dtypes.bfloat16
    f = {k: np.asarray(v, np.float64) for k, v in inputs.items()}
    LI = LAMBDA_INIT

    lam1 = np.exp(np.sum(f["lq1"] * f["lk1"]))
    lam2 = np.exp(np.sum(f["lq2"] * f["lk2"]))
    lam = lam1 - lam2 + LI
    mg, mb = f["mha_ln_g"], f["mha_ln_b"]
    Wk1 = f["k1_w"] * mg[:, None]; bk1 = mb @ f["k1_w"] + f["k1_b"]
    Wk2 = f["k2_w"] * mg[:, None]; bk2 = mb @ f["k2_w"] + f["k2_b"]
    Wv = f["v_w"] * mg[:, None]; bv = mb @ f["v_w"] + f["v_b"]
    Wq1 = f["q1_w"] * mg[:, None]; bq1 = mb @ f["q1_w"] + f["q1_b"]
    Wq2 = f["q2_w"] * mg[:, None]; bq2 = mb @ f["q2_w"] + f["q2_b"]
    Wob = f["attn_ln_g"][:, None] * f["out_w"] * (1 - LI)
    bo = (1 - LI) * (f["attn_ln_b"] @ f["out_w"]) + f["out_b"]
    W1p = f["ffn_ln_g"][:, None] * f["ffn1_w"]
    b1p = f["ffn_ln_b"] @ f["ffn1_w"] + f["ffn1_b"]

    kk = np.arange(1, 11)
    ang = f["e"][:, None] * kk / np.pi
    ne = (np.cos(ang) @ f["kan_a"] + np.sin(ang) @ f["kan_b"]
          + f["kan_bias"][0]) * f["alpha_w"][0, 0]
    ne_pad = np.zeros(NSUB * 128)
    ne_pad[:N] = ne
    ne_pm = np.ascontiguousarray(
        ne_pad.reshape(NSUB, 128).T.astype(np.float32))

    wb = np.concatenate(
        [f["fe_w1"].reshape(4, 128, 128)[i] for i in range(4)]
        + [f["fe_w2"], Wk1, Wk2, Wv, Wq1.T, -lam * Wq2.T, Wob, W1p,
           f["ffn2_w"]], axis=1)
    wb = np.ascontiguousarray(wb.astype(bf))
    colw = np.stack([f["fe_b1"], f["fe_b2"], bo, b1p, f["ffn2_b"],
                     np.zeros(128), np.zeros(128), np.zeros(128)], axis=1)
    colw = np.ascontiguousarray(colw.astype(np.float32))
    colb = np.ascontiguousarray(
        np.stack([bq1, -lam * bq2], axis=1).astype(bf))
    rowb = np.ascontiguousarray(
        np.concatenate([bk1, bk2, bv, np.zeros(128)])[None, :].astype(bf))
    roww = np.ascontiguousarray((N * bv)[None, :].astype(np.float32))

    x = np.asarray(inputs["x"], np.float32)
    u = np.asarray(inputs["u"], np.float32)
    in_maps = []
    for c in range(CORES):
        sh = u[c * NLOC:(c + 1) * NLOC]
        m = {
            "u": np.ascontiguousarray(sh.astype(bf)),
            "uT": np.ascontiguousarray(sh.T.astype(bf)),
            "xT": np.ascontiguousarray(
                x[c * NLOC:(c + 1) * NLOC].T.astype(bf)),
            "wb": wb, "colw": colw, "colb": colb, "rowb": rowb,
            "roww": roww, "ne": ne_pm,
        }
        in_maps.append(m)
    return in_maps


def assemble_out(res, CORES=CORES_FULL):
    # per-core outputs are [D, NLOC] (transposed); transpose + concat rows
    return np.concatenate(
        [np.asarray(res.results[c]["out"]).T for c in range(CORES)],
        axis=0).astype(np.float32)


def kernel(**inputs):
    from concourse import bass_utils

    nc = _get_nc()
    in_maps = make_in_maps(inputs, N_FULL, CORES_FULL)
    res = bass_utils.run_bass_kernel_spmd(nc, in_maps,
                                          core_ids=list(range(CORES_FULL)))
    return assemble_out(res)


if __name__ == "__main__":
    build_kernel()
    print("build ok")
